# revision 17
# baseline (speedup 1.0000x reference)
"""GATv2 graph net (IMDB) Trainium2 kernel — 8-core SPMD, fp16 edge phase.

Architecture (v2):
- dst-partition edges across 8 cores on graph-aligned node ranges; per-core
  degree-sorted padded ELL (rows = destination nodes, slots on the free dim).
- Node features (xl|xr) are stored fp16 in a replicated DRAM table with 256B
  rows addressed by a transposed tiling r = (g%128)*401 + g//128.  Per-slot
  source xl (64B) comes in via SWDGE dma_gather.  int16 gather indices only
  span 32768 rows, so two overlapping windows are used: [0,32768) and
  [18560,51328).  Sources whose row falls in the overlap are assigned per
  destination row to whichever window balances the two slot counts, which
  keeps per-tile ELL padding low (~1.3x vs 1.7x for a blind split).
- Edge compute runs on group-uniform slot counts: G consecutive tiles share
  one K per window so the whole group is processed by ~20 large fp16 vector
  instructions (4D access patterns, 2x DVE mode) instead of ~26 per tile.
  exp() stays f32 (fp16 would overflow); alpha is normalized before the
  weighted message sum so everything downstream of exp is fp16 again.
- ELU's -1 is folded into the next layer's bias (b2' = b2 - W2@1,
  bfc' = bfc - Wfc@1), so the edge phase emits relu(o)+exp(min(o,0)).
- Mean-pool is a matmul: pooled^T = sum_t x3_tile^T @ Sel_tile where Sel has
  1/graph_size at (node row, local graph) — no gather, no transposes, and the
  fc layer consumes pooled^T directly.
- Between convs the per-core x2^T is AllGathered in fp16.
"""

import sys

sys.path.insert(0, "/opt/trn_rl_repo")

import numpy as np

import concourse.bass as bass
import concourse.bacc as bacc
import concourse.tile as tile
from concourse import mybir
from concourse.bass import exact_div
from concourse.masks import make_identity
from concourse import bass_utils

# ---------------------------------------------------------------- constants
N = 50000
E = 1_600_000
H = 2
C = 16
HC = H * C            # 32
G = 512
NCORES = 8
GPC = G // NCORES     # graphs per core = 64
NPC = 6400            # padded nodes per core (50 tiles of 128)
NT = NPC // 128       # node tiles per core = 50
NG = NCORES * NPC     # padded global node count = 51200
TT = NG // 128        # 400 tiles in gid space
TC1 = TT + 1          # row-columns per partition (400 tiles + 1 spare)
ROWS = 128 * TC1      # node table rows = 51328
W1LO = ROWS - 32768   # window1 base row = 18560
SENT = 60 * TC1 + 400 # sentinel row (spare column, inside the overlap) = 24460+400
BIG = 49152.0         # fp16-safe saturation value for sentinel xl
BUD = 160             # group budget: T*(K0+K1) <= BUD
TMAX = 12             # max tiles per group
F32 = mybir.dt.float32
F16 = mybir.dt.float16
I32 = mybir.dt.int32
I16 = mybir.dt.int16
AX = mybir.AxisListType
ALU = mybir.AluOpType
ACTF = mybir.ActivationFunctionType


def _r_of_gid(g):
    """node-table row for gid: transposed tiling so per-partition rows of one
    core are consecutive (xr indirect read) and node-phase writes batch."""
    return (g % 128) * TC1 + g // 128


def _wrap16(flat):
    """dma_gather index layout: flat i -> [16*g + i%16, i//16], replicated
    across the 8 Q7-core partition groups."""
    flat = np.asarray(flat, np.int16)
    n = len(flat)
    assert n % 16 == 0
    arr = np.empty((128, n // 16), np.int16)
    blk = flat.reshape(n // 16, 16).T
    for g in range(8):
        arr[g * 16 : (g + 1) * 16, :] = blk
    return arr


# ================================================================ host prep
def _prep(inputs):
    ei = np.asarray(inputs["edge_index"])
    src = ei[0].astype(np.int64)
    dst = ei[1].astype(np.int64)
    batch = np.asarray(inputs["batch"]).astype(np.int64)
    rand_feat = np.asarray(inputs["rand_feat"], dtype=np.float32).reshape(-1)

    deg = (np.bincount(src, minlength=N) + np.bincount(dst, minlength=N)).astype(
        np.float32
    )
    ddeg = (np.bincount(dst, minlength=N) + 1).astype(np.int64)  # + self loop

    # graph-aligned core boundaries
    bounds = np.searchsorted(batch, np.arange(0, G + 1, GPC))
    assert bounds[0] == 0 and bounds[-1] == N
    ncs = np.diff(bounds)
    assert ncs.max() <= NPC, ncs.max()

    # per-core degree-sorted node order; gid = core*NPC + rank
    gid = np.empty(N, np.int64)
    for c in range(NCORES):
        lo, hi = bounds[c], bounds[c + 1]
        order = np.argsort(ddeg[lo:hi], kind="stable") + lo
        gid[order] = c * NPC + np.arange(hi - lo)

    # edges (+self loops)
    src_sl = np.concatenate([src, np.arange(N, dtype=np.int64)])
    dst_sl = np.concatenate([dst, np.arange(N, dtype=np.int64)])
    EP = len(src_sl)
    sr = _r_of_gid(gid[src_sl])          # source node-table row
    dgid = gid[dst_sl]

    # ---- window assignment with per-dst balancing over the overlap band
    forced1 = sr >= 32768
    forced0 = sr < W1LO
    flexm = (~forced0) & (~forced1)
    f0 = np.bincount(dgid[forced0], minlength=NG)
    f1 = np.bincount(dgid[forced1], minlength=NG)
    fx = np.bincount(dgid[flexm], minlength=NG)
    degg = f0 + f1 + fx
    quota0 = np.clip((f1 + fx - f0 + 1) // 2, 0, fx)  # flex slots -> window0
    fill0 = f0 + quota0
    fill1 = degg - fill0

    # order edges by dst gid; rank flex edges within each dst
    eorder = np.argsort(dgid, kind="stable")
    dg_s = dgid[eorder]
    sr_s = sr[eorder]
    flex_s = flexm[eorder]
    half_s = forced1[eorder].astype(np.int64)
    fidx = np.nonzero(flex_s)[0]
    dgf = dg_s[fidx]                      # sorted (dg_s sorted)
    frank = np.arange(len(fidx)) - np.searchsorted(dgf, dgf, side="left")
    half_s[fidx] = (frank >= quota0[dgf]).astype(np.int64)

    # slot index within (dst, half); slots ordered by source row so the
    # gather's descriptors walk ascending HBM addresses (row-buffer locality)
    key2 = dg_s * 2 + half_s
    k2o = np.lexsort((sr_s, key2))
    ks_start = np.searchsorted(key2[k2o], np.arange(NG * 2))
    slot = np.empty(EP, np.int64)
    slot[k2o] = np.arange(EP) - ks_start[key2[k2o]]

    # per-tile K per window (max over cores and rows)
    F0 = fill0.reshape(NCORES, NT, 128)
    F1 = fill1.reshape(NCORES, NT, 128)
    K0t = F0.max(axis=(0, 2)).astype(np.int64)
    K1t = F1.max(axis=(0, 2)).astype(np.int64)

    # greedy grouping: T consecutive tiles share (K0, K1); T*(K0+K1) <= BUD
    groups = []
    t = 0
    while t < NT:
        T = 1
        k0 = int(K0t[t]); k1 = int(K1t[t])
        while t + T < NT and T < TMAX:
            nk0 = max(k0, int(K0t[t + T])); nk1 = max(k1, int(K1t[t + T]))
            if (T + 1) * (nk0 + nk1) > BUD:
                break
            T += 1; k0, k1 = nk0, nk1
        groups.append((t, T, k0, k1))
        t += T

    colbase = np.zeros((2, NT), np.int64)   # slot-column start per (half, tile)
    goff = [0, 0]
    groups2 = []
    for (t0, T, k0, k1) in groups:
        assert k0 >= 1 and k1 >= 1
        groups2.append((t0, T, k0, k1, goff[0], goff[1]))
        for tt in range(T):
            colbase[0, t0 + tt] = goff[0] + tt * k0
            colbase[1, t0 + tt] = goff[1] + tt * k1
        goff[0] += T * k0
        goff[1] += T * k1
    groups = groups2
    ncols = [int(goff[0]), int(goff[1])]
    n_slots = (ncols[0] + ncols[1]) * 128

    # ---- ELL index arrays (wrapped int16), pads -> sentinel
    core_of = dg_s // NPC
    j = dg_s % NPC
    tt_ = j // 128
    p_ = j % 128
    ell = []
    for h in range(2):
        base = W1LO if h == 1 else 0
        sent_rel = SENT - base
        flat = np.full((NCORES, ncols[h] * 128), sent_rel, np.int64)
        selm = np.nonzero(half_s == h)[0]
        ghk = [(g[2] if h == 0 else g[3]) for g in groups]
        # group K per tile for assert
        pos = (colbase[h][tt_[selm]] + slot[selm]) * 128 + p_[selm]
        flat[core_of[selm], pos] = sr_s[selm] - base
        assert flat.max() <= 32767 and flat.min() >= 0
        ell.append(flat)

    # ---- pooling selection matrix (1/graph_size at (row, local graph))
    gsz = np.bincount(batch, minlength=G).astype(np.float64)
    sel = np.zeros((NCORES, 128, NT * GPC), np.float16)
    # node n sits at gid[n] = c*NPC + j ; local graph = batch[n] - c*GPC
    nc_ = gid // NPC
    nj = gid % NPC
    npp = nj % 128
    ntt = nj // 128
    lg = batch - nc_ * GPC
    sel[nc_, npp, ntt * GPC + lg] = (1.0 / np.maximum(gsz[batch], 1.0)).astype(
        np.float16
    )

    # ---- x1 features in gid-column order [2, NG]
    x1feat = np.zeros((2, NG), np.float32)
    x1feat[0, gid] = deg
    x1feat[1, gid] = rand_feat

    # ---- own-row base for the per-conv xr indirect read
    own_base = np.empty((NCORES, 128, 1), np.int32)
    for c in range(NCORES):
        own_base[c, :, 0] = np.arange(128) * TC1 + c * NT

    # ---- packed weights
    def f32(x):
        return np.asarray(x, dtype=np.float32)

    W1l, W1r = f32(inputs["W1l"]), f32(inputs["W1r"])
    b1 = np.concatenate([f32(inputs["b1l"]), f32(inputs["b1r"])])
    W2l, W2r = f32(inputs["W2l"]), f32(inputs["W2r"])
    # fold ELU's -1 (x2_stored = x2_true + 1) into conv2 bias
    b2 = np.concatenate(
        [f32(inputs["b2l"]) - W2l.sum(axis=1), f32(inputs["b2r"]) - W2r.sum(axis=1)]
    )
    att1 = f32(inputs["att1"]).reshape(HC)
    att2 = f32(inputs["att2"]).reshape(HC)
    bias1 = f32(inputs["bias1"])
    bias2 = f32(inputs["bias2"])
    Wfc, bfc = f32(inputs["Wfc"]), f32(inputs["bfc"])
    bfc = bfc - Wfc.sum(axis=1)  # fold x3_stored = x3_true + 1

    wpack1 = np.concatenate([W1l.T, W1r.T], axis=1)
    wpack1[0, :] += b1
    wpack2 = np.concatenate([W2l.T, W2r.T], axis=1)
    wpack2 = np.concatenate([wpack2, b2[None, :]], axis=0).astype(np.float16)

    shared = dict(
        x1feat=x1feat,
        wpack1=wpack1,
        wpack2=wpack2,
        att1_rep=np.tile(att1[None, :], (128, 1)).astype(np.float16),
        att2_rep=np.tile(att2[None, :], (128, 1)).astype(np.float16),
        bias1_rep=np.tile(bias1[None, :], (128, 1)).astype(np.float32),
        bias2_rep=np.tile(bias2[None, :], (128, 1)).astype(np.float32),
        padrow1=(-BIG * np.sign(att1)[None, :]).astype(np.float16),
        padrow2=(-BIG * np.sign(att2)[None, :]).astype(np.float16),
        wfcT=Wfc.T.astype(np.float16).copy(),
        bfc_rep=np.tile(bfc[None, :], (GPC, 1)).astype(np.float32),
    )
    in_maps = []
    for c in range(NCORES):
        m = dict(shared)
        m["ell0"] = _wrap16(ell[0][c])
        m["ell1"] = _wrap16(ell[1][c])
        m["sel"] = np.ascontiguousarray(sel[c])
        m["own_base"] = np.ascontiguousarray(own_base[c])
        in_maps.append(m)

    cfg = dict(groups=tuple(groups), ncols=(ncols[0], ncols[1]), n_slots=n_slots)
    return cfg, in_maps


def _cfg_key(cfg):
    return cfg["groups"]


# ============================================================= device build
def dma_gather_raw(gp, out_ap, in_ap, idxs_ap, num_idxs, elem_size, elem_step,
                   queue_num=0):
    """dma_gather without the elem_size%256 assert (64B elems work on HW)."""
    stride_bytes = elem_step * mybir.dt.size(in_ap.dtype)
    stride_bytes_256 = exact_div(stride_bytes, 256)
    _in_ap = gp.lower_ap_dma(in_ap, for_custom_bir_dma=True)
    _idxs_ap = gp.lower_ap(idxs_ap)
    _out_ap = gp.lower_ap(out_ap)
    return gp.add_instruction(
        mybir.InstDMAGatherAnt(
            name=gp.bass.get_next_instruction_name(),
            ins=[*_in_ap, _idxs_ap, gp.lower_val_access(gp.to_reg(num_idxs))],
            outs=[_out_ap],
            transpose=False,
            num_idxs=num_idxs,
            elem_size=elem_size,
            stride_bytes_256=stride_bytes_256,
            gen_mode=0,
            single_packet=False,
            queue_num=queue_num,
        )
    )


def _node_phase_conv1(nc, tc, ctx, x1feat, wp1_t, nodefeat, padrow1):
    A = 8
    with tc.tile_pool(name="n1sb", bufs=3) as sb, tc.tile_pool(
        name="n1ps", bufs=3, space="PSUM"
    ) as ps:
        for g0 in range(0, TT, A):
            x1t = sb.tile([3, A * 128], F32, tag="x1t")
            nc.gpsimd.memset(x1t[0:1, :], 1.0)
            nc.sync.dma_start(
                out=x1t[1:3, :], in_=x1feat[:, g0 * 128 : (g0 + A) * 128]
            )
            stage = sb.tile([128, A * 64], F16, tag="n1stage")
            for a4 in range(0, A, 4):
                pt = ps.tile([128, 4 * 64], F32, tag="n1psum")
                for a in range(4):
                    nc.tensor.matmul(
                        out=pt[:, a * 64 : (a + 1) * 64],
                        lhsT=x1t[:, (a4 + a) * 128 : (a4 + a + 1) * 128],
                        rhs=wp1_t[:],
                        start=True,
                        stop=True,
                    )
                nc.scalar.copy(out=stage[:, a4 * 64 : (a4 + 4) * 64], in_=pt[:])
            dst = bass.AP(
                nodefeat[:].tensor,
                g0 * 128,
                [[TC1 * 128, 128], [128, A], [1, 64]],
            )
            nc.sync.dma_start(out=dst, in_=stage[:])
        pr = sb.tile([1, HC], F16, tag="n1pad")
        nc.sync.dma_start(out=pr[:], in_=padrow1[:])
        nc.sync.dma_start(out=nodefeat[SENT : SENT + 1, 0:HC], in_=pr[:])


def _node_phase_conv2(nc, tc, ctx, agout, wp2_t, nodefeat, padrow2):
    A = 5
    with tc.tile_pool(name="n2sb", bufs=3) as sb, tc.tile_pool(
        name="n2ps", bufs=3, space="PSUM"
    ) as ps:
        for c8 in range(NCORES):
            for t0 in range(0, NT, A):
                lh = sb.tile([33, A * 128], F16, tag="n2lhs")
                src_ap = bass.AP(
                    agout[:].tensor,
                    c8 * 33 * NPC + t0 * 128,
                    [[NPC, 33], [1, A * 128]],
                )
                nc.sync.dma_start(out=lh[:], in_=src_ap)
                stage = sb.tile([128, A * 64], F16, tag="n2stage")
                pt = ps.tile([128, A * 64], F32, tag="n2psum")
                for a in range(A):
                    nc.tensor.matmul(
                        out=pt[:, a * 64 : (a + 1) * 64],
                        lhsT=lh[:, a * 128 : (a + 1) * 128],
                        rhs=wp2_t[:],
                        start=True,
                        stop=True,
                    )
                nc.scalar.copy(out=stage[:], in_=pt[:])
                dst = bass.AP(
                    nodefeat[:].tensor,
                    (c8 * NT + t0) * 128,
                    [[TC1 * 128, 128], [128, A], [1, 64]],
                )
                nc.sync.dma_start(out=dst, in_=stage[:])
        pr = sb.tile([1, HC], F16, tag="n2pad")
        nc.sync.dma_start(out=pr[:], in_=padrow2[:])
        nc.sync.dma_start(out=nodefeat[SENT : SENT + 1, 0:HC], in_=pr[:])


def _edge_phase(nc, tc, ctx, cfg, conv, nodefeat, ell_d, att_t, bias_t, own_base_t,
                x2t_stage, x3_t, sel_t, pool_ps, identity_t):
    """One GATv2 conv aggregation over group-uniform ELL tiles.

    conv=1: writes transposed x2 into x2t_stage [33, NPC] (fp16).
    conv=2: writes x3 tiles into x3_t [128, NT*HC] (fp16) and accumulates the
            pooling matmul into pool_ps [HC, GPC].
    """
    groups = cfg["groups"]
    MAXH = max(max(T * k0, T * k1) for (_, T, k0, k1, _, _) in groups)
    MAXT = max(T for (_, T, k0, k1, _, _) in groups)
    sbg = ctx.enter_context(tc.tile_pool(name=f"e{conv}sbg", bufs=6))
    sbi = ctx.enter_context(tc.tile_pool(name=f"e{conv}sbi", bufs=6))
    sbz = ctx.enter_context(tc.tile_pool(name=f"e{conv}sbz", bufs=2))
    sbm = ctx.enter_context(tc.tile_pool(name=f"e{conv}sbm", bufs=3))
    sbs = ctx.enter_context(tc.tile_pool(name=f"e{conv}sbs", bufs=2))
    ps = ctx.enter_context(tc.tile_pool(name=f"e{conv}ps", bufs=2, space="PSUM"))

    # all own-node rows (xl|xr fp16, full 256B rows) for this core
    xrp = ctx.enter_context(tc.tile_pool(name=f"e{conv}xr", bufs=1))
    xrall = xrp.tile([128, NT * 128], F16, name=f"xrall{conv}")
    nc.gpsimd.indirect_dma_start(
        out=xrall[:],
        out_offset=None,
        in_=nodefeat[:],
        in_offset=bass.IndirectOffsetOnAxis(ap=own_base_t[:], axis=0),
        element_offset=0,
    )

    def ap4(t, off, dims):
        return bass.AP(t[:].tensor, t[:].offset + off, [t[:].ap[0]] + dims)

    pdim = xrall[:].ap[0]
    NGRP = len(groups)
    for gi in range(NGRP - 1, -1, -1):          # big groups first
        t0, T, gk0, gk1, off0, off1 = groups[gi]
        gk = (gk0, gk1)
        goff = (off0, off1)
        gbuf = [None, None]
        for h in range(2):
            nidx = 128 * T * gk[h]
            i0 = goff[h] * 128
            idx_t = sbi.tile([128, nidx // 16], I16, tag=f"idx{h}",
                             padded_shape=[128, MAXH * 8])
            nc.sync.dma_start(
                out=idx_t[:], in_=ell_d[h][:, i0 // 16 : (i0 + nidx) // 16]
            )
            gbuf[h] = sbg.tile(
                [128, T * gk[h] * HC], F16, tag=f"g{h}", name=f"gbuf{h}",
                padded_shape=[128, MAXH * HC],
            )
            base = W1LO if h == 1 else 0
            dma_gather_raw(
                nc.gpsimd,
                gbuf[h][:].rearrange("p (n e) -> p n e", e=HC),
                nodefeat[base : base + 32768, 0:HC],
                idx_t[:],
                nidx,
                HC,
                128,
                queue_num=0,
            )

        # per-half chain: z -> lrelu -> *att -> score -> exp -> msg (frees gbuf)
        ph = [None, None]
        den_h = [None, None]
        nh = [None, None]
        for h in range(2):
            TK = T * gk[h]
            z_t = sbz.tile([128, TK * HC], F16, tag="z",
                           padded_shape=[128, MAXH * HC])
            nc.vector.tensor_tensor(
                out=z_t[:],
                in0=gbuf[h][:],
                in1=bass.AP(xrall[:].tensor, xrall[:].offset + t0 * 128 + HC,
                            [pdim, [128, T], [0, gk[h]], [1, HC]]),
                op=ALU.add,
            )
            e_t = sbz.tile([128, TK * HC], F16, tag="e",
                           padded_shape=[128, MAXH * HC])
            nc.vector.scalar_tensor_tensor(
                out=e_t[:], in0=z_t[:], scalar=0.2, in1=z_t[:],
                op0=ALU.mult, op1=ALU.max,
            )
            ea_t = sbz.tile([128, TK * HC], F16, tag="ea",
                            padded_shape=[128, MAXH * HC])
            nc.vector.tensor_tensor(
                out=ea_t[:],
                in0=e_t[:],
                in1=bass.AP(att_t[:].tensor, att_t[:].offset,
                            [att_t[:].ap[0], [0, TK], [1, HC]]),
                op=ALU.mult,
            )
            s_t = sbs.tile([128, TK * H], F32, tag="s",
                           padded_shape=[128, MAXH * H])
            nc.vector.tensor_reduce(
                out=s_t[:],
                in_=ap4(ea_t, 0, [[HC, TK], [C, H], [1, C]]),
                axis=AX.X,
                op=ALU.add,
            )
            sc_t = sbs.tile([128, TK * H], F32, tag="sc",
                            padded_shape=[128, MAXH * H])
            nc.vector.tensor_scalar(
                out=sc_t[:], in0=s_t[:], scalar1=-80.0, scalar2=None,
                op0=ALU.max,
            )
            p_t = sbs.tile([128, TK * H], F32, tag="p", name=f"p{h}",
                           padded_shape=[128, MAXH * H])
            nc.scalar.activation(p_t[:], sc_t[:], ACTF.Exp)
            ph[h] = p_t
            den_h[h] = sbs.tile([128, T * H], F32, tag="den", name=f"den{h}",
                                padded_shape=[128, MAXT * H])
            nc.vector.tensor_reduce(
                out=den_h[h][:],
                in_=ap4(p_t, 0, [[gk[h] * H, T], [1, H], [H, gk[h]]]),
                axis=AX.X,
                op=ALU.add,
            )
        den_t = sbs.tile([128, T * H], F32, tag="dens",
                         padded_shape=[128, MAXT * H])
        nc.vector.tensor_add(out=den_t[:], in0=den_h[0][:], in1=den_h[1][:])
        rden_t = sbs.tile([128, T * H], F32, tag="rden",
                          padded_shape=[128, MAXT * H])
        nc.vector.reciprocal(out=rden_t[:], in_=den_t[:])
        # alpha = p*rden (fp16, per half) ; msg = xl*alpha ; num_h = sum_k msg
        for h in range(2):
            TK = T * gk[h]
            al_t = sbs.tile([128, TK * H], F16, tag="al", name=f"al{h}",
                            padded_shape=[128, MAXH * H])
            nc.vector.tensor_tensor(
                out=al_t[:],
                in0=ph[h][:],
                in1=ap4(rden_t, 0, [[H, T], [0, gk[h]], [1, H]]),
                op=ALU.mult,
            )
            msg = sbm.tile([128, TK * HC], F16, tag="m",
                           padded_shape=[128, MAXH * HC])
            nc.vector.tensor_tensor(
                out=msg[:],
                in0=gbuf[h][:],
                in1=ap4(al_t, 0, [[H, TK], [1, H], [0, C]]),
                op=ALU.mult,
            )
            nh[h] = sbs.tile([128, T * HC], F32, tag=f"n{h}", name=f"nh{h}",
                             padded_shape=[128, MAXT * HC])
            nc.vector.tensor_reduce(
                out=nh[h][:],
                in_=ap4(msg, 0, [[gk[h] * HC, T], [1, HC], [HC, gk[h]]]),
                axis=AX.X,
                op=ALU.add,
            )
        num_t = sbs.tile([128, T * HC], F32, tag="num",
                         padded_shape=[128, MAXT * HC])
        nc.vector.tensor_add(out=num_t[:], in0=nh[0][:], in1=nh[1][:])
        # o = num + bias ; x2 = relu(o) + exp(clamp(min(o,0),-80)) (+1 folded)
        o_t = sbs.tile([128, T * HC], F32, tag="o",
                       padded_shape=[128, MAXT * HC])
        nc.vector.tensor_tensor(
            out=o_t[:],
            in0=num_t[:],
            in1=bass.AP(bias_t[:].tensor, bias_t[:].offset,
                        [bias_t[:].ap[0], [0, T], [1, HC]]),
            op=ALU.add,
        )
        mn_t = sbs.tile([128, T * HC], F32, tag="mn",
                        padded_shape=[128, MAXT * HC])
        nc.vector.tensor_scalar(
            out=mn_t[:], in0=o_t[:], scalar1=0.0, scalar2=-80.0,
            op0=ALU.min, op1=ALU.max,
        )
        ex_t = sbs.tile([128, T * HC], F32, tag="ex",
                        padded_shape=[128, MAXT * HC])
        nc.scalar.activation(ex_t[:], mn_t[:], ACTF.Exp)
        if conv == 1:
            x2g = sbs.tile([128, T * HC], F16, tag="x2g",
                           padded_shape=[128, MAXT * HC])
            nc.vector.scalar_tensor_tensor(
                out=x2g[:], in0=o_t[:], scalar=0.0, in1=ex_t[:],
                op0=ALU.max, op1=ALU.add,
            )
            # transpose into x2t_stage [33, NPC] via PE, 4 tiles per transpose
            for ch0 in range(0, T, 4):
                cw = min(4, T - ch0) * HC
                tp = ps.tile([128, 128], F16, tag="tp")
                nc.tensor.transpose(
                    out=tp[0:cw, :],
                    in_=x2g[:, ch0 * HC : ch0 * HC + cw],
                    identity=identity_t[:],
                )
                for k in range((cw) // HC):
                    tt = t0 + ch0 + k
                    nc.scalar.copy(
                        out=x2t_stage[0:HC, tt * 128 : (tt + 1) * 128],
                        in_=tp[k * HC : (k + 1) * HC, :],
                    )
        else:
            nc.vector.scalar_tensor_tensor(
                out=x3_t[:, t0 * HC : (t0 + T) * HC], in0=o_t[:], scalar=0.0,
                in1=ex_t[:], op0=ALU.max, op1=ALU.add,
            )
            first_exec = groups[-1][0]
            last_exec = groups[0][0] + groups[0][1] - 1
            for k in range(T):
                tt = t0 + k
                nc.tensor.matmul(
                    out=pool_ps[:],
                    lhsT=x3_t[:, tt * HC : (tt + 1) * HC],
                    rhs=sel_t[:, tt * GPC : (tt + 1) * GPC],
                    start=(tt == first_exec),
                    stop=(tt == last_exec),
                )


def _build(cfg):
    nc = bacc.Bacc("TRN2", target_bir_lowering=False, debug=False,
                   num_devices=NCORES)
    ncol0, ncol1 = cfg["ncols"]

    x1feat = nc.dram_tensor("x1feat", [2, NG], F32, kind="ExternalInput").ap()
    ell0 = nc.dram_tensor("ell0", [128, ncol0 * 8], I16, kind="ExternalInput").ap()
    ell1 = nc.dram_tensor("ell1", [128, ncol1 * 8], I16, kind="ExternalInput").ap()
    sel = nc.dram_tensor("sel", [128, NT * GPC], F16, kind="ExternalInput").ap()
    own_base = nc.dram_tensor("own_base", [128, 1], I32, kind="ExternalInput").ap()
    wpack1 = nc.dram_tensor("wpack1", [3, 64], F32, kind="ExternalInput").ap()
    wpack2 = nc.dram_tensor("wpack2", [33, 64], F16, kind="ExternalInput").ap()
    att1_rep = nc.dram_tensor("att1_rep", [128, HC], F16, kind="ExternalInput").ap()
    att2_rep = nc.dram_tensor("att2_rep", [128, HC], F16, kind="ExternalInput").ap()
    bias1_rep = nc.dram_tensor("bias1_rep", [128, HC], F32, kind="ExternalInput").ap()
    bias2_rep = nc.dram_tensor("bias2_rep", [128, HC], F32, kind="ExternalInput").ap()
    padrow1 = nc.dram_tensor("padrow1", [1, HC], F16, kind="ExternalInput").ap()
    padrow2 = nc.dram_tensor("padrow2", [1, HC], F16, kind="ExternalInput").ap()
    wfcT = nc.dram_tensor("wfcT", [HC, 2], F16, kind="ExternalInput").ap()
    bfc_rep = nc.dram_tensor("bfc_rep", [GPC, 2], F32, kind="ExternalInput").ap()
    logits_out = nc.dram_tensor("logits", [GPC, 2], F32, kind="ExternalOutput").ap()

    with tile.TileContext(nc) as tc:
        from contextlib import ExitStack

        with ExitStack() as top:
            dram = top.enter_context(tc.tile_pool(name="dram", bufs=1, space="DRAM"))
            nodefeat1 = dram.tile([ROWS, 128], F16)
            nodefeat2 = dram.tile([ROWS, 128], F16)
            agin = dram.tile([33, NPC], F16)
            agout = dram.tile([NCORES * 33, NPC], F16)

            consts = top.enter_context(tc.tile_pool(name="consts", bufs=1))
            wp1_t = consts.tile([3, 64], F32)
            nc.sync.dma_start(out=wp1_t[:], in_=wpack1[:])
            wp2_t = consts.tile([33, 64], F16)
            nc.sync.dma_start(out=wp2_t[:], in_=wpack2[:])
            att1_t = consts.tile([128, HC], F16)
            nc.sync.dma_start(out=att1_t[:], in_=att1_rep[:])
            att2_t = consts.tile([128, HC], F16)
            nc.sync.dma_start(out=att2_t[:], in_=att2_rep[:])
            bias1_t = consts.tile([128, HC], F32)
            nc.sync.dma_start(out=bias1_t[:], in_=bias1_rep[:])
            bias2_t = consts.tile([128, HC], F32)
            nc.sync.dma_start(out=bias2_t[:], in_=bias2_rep[:])
            idf32 = consts.tile([128, 128], F32)
            make_identity(nc, idf32[:])
            identity_t = consts.tile([128, 128], F16)
            nc.vector.tensor_copy(out=identity_t[:], in_=idf32[:])
            own_base_t = consts.tile([128, 1], I32)
            nc.sync.dma_start(out=own_base_t[:], in_=own_base[:])
            sel_t = consts.tile([128, NT * GPC], F16)
            nc.sync.dma_start(out=sel_t[:], in_=sel[:])

            # ---------------- conv1
            _node_phase_conv1(nc, tc, top, x1feat, wp1_t, nodefeat1, padrow1)

            with ExitStack() as c1:
                stage_pool = c1.enter_context(tc.tile_pool(name="x2tst", bufs=1))
                x2t_stage = stage_pool.tile([33, NPC], F16)
                nc.gpsimd.memset(x2t_stage[32:33, :], 1.0)
                _edge_phase(nc, tc, c1, cfg, 1, nodefeat1, (ell0, ell1),
                            att1_t, bias1_t, own_base_t, x2t_stage, None, None,
                            None, identity_t)
                nc.sync.dma_start(out=agin[:], in_=x2t_stage[:])

            nc.gpsimd.collective_compute(
                "AllGather",
                ALU.bypass,
                replica_groups=[list(range(NCORES))],
                ins=[agin[:].opt()],
                outs=[agout[:].opt()],
            )

            # ---------------- conv2
            _node_phase_conv2(nc, tc, top, agout, wp2_t, nodefeat2, padrow2)

            with ExitStack() as c2:
                x3p = c2.enter_context(tc.tile_pool(name="x3p", bufs=1))
                x3_t = x3p.tile([128, NT * HC], F16)
                pps = c2.enter_context(tc.tile_pool(name="poolps", bufs=1,
                                                    space="PSUM"))
                pool_ps = pps.tile([HC, GPC], F32)
                _edge_phase(nc, tc, c2, cfg, 2, nodefeat2, (ell0, ell1),
                            att2_t, bias2_t, own_base_t, None, x3_t, sel_t,
                            pool_ps, identity_t)

                # ---------------- fc + log_softmax
                sb = c2.enter_context(tc.tile_pool(name="fcsb", bufs=1))
                ps2 = c2.enter_context(tc.tile_pool(name="fcps", bufs=1,
                                                    space="PSUM"))
                wfc_t = sb.tile([HC, 2], F16)
                nc.sync.dma_start(out=wfc_t[:], in_=wfcT[:])
                bfc_t = sb.tile([GPC, 2], F32)
                nc.sync.dma_start(out=bfc_t[:], in_=bfc_rep[:])
                pooledT_t = sb.tile([HC, GPC], F16)
                nc.scalar.copy(out=pooledT_t[:], in_=pool_ps[:])
                lg_ps = ps2.tile([GPC, 2], F32)
                nc.tensor.matmul(out=lg_ps[:], lhsT=pooledT_t[:],
                                 rhs=wfc_t[:], start=True, stop=True)
                lg_t = sb.tile([GPC, 2], F32)
                nc.vector.tensor_add(out=lg_t[:], in0=lg_ps[:], in1=bfc_t[:])
                mx_t = sb.tile([GPC, 1], F32)
                nc.vector.tensor_reduce(out=mx_t[:], in_=lg_t[:], axis=AX.X,
                                        op=ALU.max)
                sh_t = sb.tile([GPC, 2], F32)
                nc.vector.tensor_scalar(
                    out=sh_t[:], in0=lg_t[:], scalar1=mx_t[:, 0:1],
                    scalar2=None, op0=ALU.subtract,
                )
                exl_t = sb.tile([GPC, 2], F32)
                nc.scalar.activation(exl_t[:], sh_t[:], ACTF.Exp)
                se_t = sb.tile([GPC, 1], F32)
                nc.vector.tensor_reduce(out=se_t[:], in_=exl_t[:], axis=AX.X,
                                        op=ALU.add)
                ln_t = sb.tile([GPC, 1], F32)
                nc.scalar.activation(ln_t[:], se_t[:], ACTF.Ln)
                out_t = sb.tile([GPC, 2], F32)
                nc.vector.tensor_scalar(
                    out=out_t[:], in0=sh_t[:], scalar1=ln_t[:, 0:1],
                    scalar2=None, op0=ALU.subtract,
                )
                nc.sync.dma_start(out=logits_out[:], in_=out_t[:])

    nc.compile()
    return nc


# =================================================================== driver
_CACHE = {}


def kernel(**inputs) -> np.ndarray:
    cfg, in_maps = _prep(inputs)
    key = _cfg_key(cfg)
    if key not in _CACHE:
        _CACHE[key] = _build(cfg)
    nc = _CACHE[key]
    res = bass_utils.run_bass_kernel_spmd(nc, in_maps, core_ids=list(range(NCORES)))
    out = np.concatenate([res.results[c]["logits"] for c in range(NCORES)], axis=0)
    return out.astype(np.float32)


# revision 20
# speedup vs baseline: 1.2787x; 1.2787x over previous
"""GATv2 graph net (IMDB) Trainium2 kernel — 8-core SPMD, fp16 edge phase.

Architecture (v2):
- dst-partition edges across 8 cores on graph-aligned node ranges; per-core
  degree-sorted padded ELL (rows = destination nodes, slots on the free dim).
- Node features (xl|xr) are stored fp16 in a replicated DRAM table with 256B
  rows addressed by a transposed tiling r = (g%128)*401 + g//128.  Per-slot
  source xl (64B) comes in via SWDGE dma_gather.  int16 gather indices only
  span 32768 rows, so two overlapping windows are used: [0,32768) and
  [18560,51328).  Sources whose row falls in the overlap are assigned per
  destination row to whichever window balances the two slot counts, which
  keeps per-tile ELL padding low (~1.3x vs 1.7x for a blind split).
- Edge compute runs on group-uniform slot counts: G consecutive tiles share
  one K per window so the whole group is processed by ~20 large fp16 vector
  instructions (4D access patterns, 2x DVE mode) instead of ~26 per tile.
  exp() stays f32 (fp16 would overflow); alpha is normalized before the
  weighted message sum so everything downstream of exp is fp16 again.
- ELU's -1 is folded into the next layer's bias (b2' = b2 - W2@1,
  bfc' = bfc - Wfc@1), so the edge phase emits relu(o)+exp(min(o,0)).
- Mean-pool is a matmul: pooled^T = sum_t x3_tile^T @ Sel_tile where Sel has
  1/graph_size at (node row, local graph) — no gather, no transposes, and the
  fc layer consumes pooled^T directly.
- Between convs the per-core x2^T is AllGathered in fp16.
"""

import sys

sys.path.insert(0, "/opt/trn_rl_repo")

import numpy as np

import concourse.bass as bass
import concourse.bacc as bacc
import concourse.tile as tile
from concourse import mybir
from concourse.bass import exact_div
from concourse.masks import make_identity
from concourse import bass_utils

# ---------------------------------------------------------------- constants
N = 50000
E = 1_600_000
H = 2
C = 16
HC = H * C            # 32
G = 512
NCORES = 8
GPC = G // NCORES     # graphs per core = 64
NPC = 6400            # padded nodes per core (50 tiles of 128)
NT = NPC // 128       # node tiles per core = 50
NG = NCORES * NPC     # padded global node count = 51200
TT = NG // 128        # 400 tiles in gid space
TC1 = TT + 1          # row-columns per partition (400 tiles + 1 spare)
ROWS = 128 * TC1      # node table rows = 51328
W1LO = ROWS - 32768   # window1 base row = 18560
SENT = 60 * TC1 + 400 # sentinel row (spare column, inside the overlap) = 24460+400
BIG = 49152.0         # fp16-safe saturation value for sentinel xl
BUD = 96              # group budget: T*(K0+K1) <= BUD
TMAX = 12             # max tiles per group
F32 = mybir.dt.float32
F16 = mybir.dt.float16
I32 = mybir.dt.int32
I16 = mybir.dt.int16
AX = mybir.AxisListType
ALU = mybir.AluOpType
ACTF = mybir.ActivationFunctionType


def _r_of_gid(g):
    """node-table row for gid: transposed tiling so per-partition rows of one
    core are consecutive (xr indirect read) and node-phase writes batch."""
    return (g % 128) * TC1 + g // 128


def _wrap16(flat):
    """dma_gather index layout: flat i -> [16*g + i%16, i//16], replicated
    across the 8 Q7-core partition groups."""
    flat = np.asarray(flat, np.int16)
    n = len(flat)
    assert n % 16 == 0
    arr = np.empty((128, n // 16), np.int16)
    blk = flat.reshape(n // 16, 16).T
    for g in range(8):
        arr[g * 16 : (g + 1) * 16, :] = blk
    return arr


# ================================================================ host prep
def _prep(inputs):
    ei = np.asarray(inputs["edge_index"])
    src = ei[0].astype(np.int64)
    dst = ei[1].astype(np.int64)
    batch = np.asarray(inputs["batch"]).astype(np.int64)
    rand_feat = np.asarray(inputs["rand_feat"], dtype=np.float32).reshape(-1)

    deg = (np.bincount(src, minlength=N) + np.bincount(dst, minlength=N)).astype(
        np.float32
    )
    ddeg = (np.bincount(dst, minlength=N) + 1).astype(np.int64)  # + self loop

    # graph-aligned core boundaries
    bounds = np.searchsorted(batch, np.arange(0, G + 1, GPC))
    assert bounds[0] == 0 and bounds[-1] == N
    ncs = np.diff(bounds)
    assert ncs.max() <= NPC, ncs.max()

    # per-core degree-sorted node order; gid = core*NPC + rank
    gid = np.empty(N, np.int64)
    for c in range(NCORES):
        lo, hi = bounds[c], bounds[c + 1]
        order = np.argsort(ddeg[lo:hi], kind="stable") + lo
        gid[order] = c * NPC + np.arange(hi - lo)

    # edges (+self loops)
    src_sl = np.concatenate([src, np.arange(N, dtype=np.int64)])
    dst_sl = np.concatenate([dst, np.arange(N, dtype=np.int64)])
    EP = len(src_sl)
    sr = _r_of_gid(gid[src_sl])          # source node-table row
    dgid = gid[dst_sl]

    # ---- window assignment with per-dst balancing over the overlap band
    forced1 = sr >= 32768
    forced0 = sr < W1LO
    flexm = (~forced0) & (~forced1)
    f0 = np.bincount(dgid[forced0], minlength=NG)
    f1 = np.bincount(dgid[forced1], minlength=NG)
    fx = np.bincount(dgid[flexm], minlength=NG)
    degg = f0 + f1 + fx
    quota0 = np.clip((f1 + fx - f0 + 1) // 2, 0, fx)  # flex slots -> window0
    fill0 = f0 + quota0
    fill1 = degg - fill0

    # order edges by dst gid; rank flex edges within each dst
    eorder = np.argsort(dgid, kind="stable")
    dg_s = dgid[eorder]
    sr_s = sr[eorder]
    flex_s = flexm[eorder]
    half_s = forced1[eorder].astype(np.int64)
    fidx = np.nonzero(flex_s)[0]
    dgf = dg_s[fidx]                      # sorted (dg_s sorted)
    frank = np.arange(len(fidx)) - np.searchsorted(dgf, dgf, side="left")
    half_s[fidx] = (frank >= quota0[dgf]).astype(np.int64)

    # slot index within (dst, half); slots ordered by source row so the
    # gather's descriptors walk ascending HBM addresses (row-buffer locality)
    key2 = dg_s * 2 + half_s
    k2o = np.lexsort((sr_s, key2))
    ks_start = np.searchsorted(key2[k2o], np.arange(NG * 2))
    slot = np.empty(EP, np.int64)
    slot[k2o] = np.arange(EP) - ks_start[key2[k2o]]

    # per-tile K per window (max over cores and rows)
    F0 = fill0.reshape(NCORES, NT, 128)
    F1 = fill1.reshape(NCORES, NT, 128)
    K0t = F0.max(axis=(0, 2)).astype(np.int64)
    K1t = F1.max(axis=(0, 2)).astype(np.int64)

    # greedy grouping: T consecutive tiles share (K0, K1); T*(K0+K1) <= BUD
    groups = []
    t = 0
    while t < NT:
        T = 1
        k0 = int(K0t[t]); k1 = int(K1t[t])
        while t + T < NT and T < TMAX:
            nk0 = max(k0, int(K0t[t + T])); nk1 = max(k1, int(K1t[t + T]))
            if (T + 1) * (nk0 + nk1) > BUD:
                break
            T += 1; k0, k1 = nk0, nk1
        groups.append((t, T, k0, k1))
        t += T

    colbase = np.zeros((2, NT), np.int64)   # slot-column start per (half, tile)
    goff = [0, 0]
    groups2 = []
    for (t0, T, k0, k1) in groups:
        assert k0 >= 1 and k1 >= 1
        groups2.append((t0, T, k0, k1, goff[0], goff[1]))
        for tt in range(T):
            colbase[0, t0 + tt] = goff[0] + tt * k0
            colbase[1, t0 + tt] = goff[1] + tt * k1
        goff[0] += T * k0
        goff[1] += T * k1
    groups = groups2
    ncols = [int(goff[0]), int(goff[1])]
    n_slots = (ncols[0] + ncols[1]) * 128

    # ---- ELL index arrays (wrapped int16), pads -> sentinel
    core_of = dg_s // NPC
    j = dg_s % NPC
    tt_ = j // 128
    p_ = j % 128
    ell = []
    for h in range(2):
        base = W1LO if h == 1 else 0
        sent_rel = SENT - base
        flat = np.full((NCORES, ncols[h] * 128), sent_rel, np.int64)
        selm = np.nonzero(half_s == h)[0]
        ghk = [(g[2] if h == 0 else g[3]) for g in groups]
        # group K per tile for assert
        pos = (colbase[h][tt_[selm]] + slot[selm]) * 128 + p_[selm]
        flat[core_of[selm], pos] = sr_s[selm] - base
        assert flat.max() <= 32767 and flat.min() >= 0
        ell.append(flat)

    # ---- pooling selection matrix (1/graph_size at (row, local graph))
    gsz = np.bincount(batch, minlength=G).astype(np.float64)
    sel = np.zeros((NCORES, 128, NT * GPC), np.float16)
    # node n sits at gid[n] = c*NPC + j ; local graph = batch[n] - c*GPC
    nc_ = gid // NPC
    nj = gid % NPC
    npp = nj % 128
    ntt = nj // 128
    lg = batch - nc_ * GPC
    sel[nc_, npp, ntt * GPC + lg] = (1.0 / np.maximum(gsz[batch], 1.0)).astype(
        np.float16
    )

    # ---- x1 features in gid-column order [2, NG]
    x1feat = np.zeros((2, NG), np.float32)
    x1feat[0, gid] = deg
    x1feat[1, gid] = rand_feat

    # ---- own-row base for the per-conv xr indirect read
    own_base = np.empty((NCORES, 128, 1), np.int32)
    for c in range(NCORES):
        own_base[c, :, 0] = np.arange(128) * TC1 + c * NT

    # ---- packed weights
    def f32(x):
        return np.asarray(x, dtype=np.float32)

    W1l, W1r = f32(inputs["W1l"]), f32(inputs["W1r"])
    b1 = np.concatenate([f32(inputs["b1l"]), f32(inputs["b1r"])])
    W2l, W2r = f32(inputs["W2l"]), f32(inputs["W2r"])
    # fold ELU's -1 (x2_stored = x2_true + 1) into conv2 bias
    b2 = np.concatenate(
        [f32(inputs["b2l"]) - W2l.sum(axis=1), f32(inputs["b2r"]) - W2r.sum(axis=1)]
    )
    att1 = f32(inputs["att1"]).reshape(HC)
    att2 = f32(inputs["att2"]).reshape(HC)
    bias1 = f32(inputs["bias1"])
    bias2 = f32(inputs["bias2"])
    Wfc, bfc = f32(inputs["Wfc"]), f32(inputs["bfc"])
    bfc = bfc - Wfc.sum(axis=1)  # fold x3_stored = x3_true + 1

    wpack1 = np.concatenate([W1l.T, W1r.T], axis=1)
    wpack1[0, :] += b1
    wpack2 = np.concatenate([W2l.T, W2r.T], axis=1)
    wpack2 = np.concatenate([wpack2, b2[None, :]], axis=0).astype(np.float16)

    shared = dict(
        x1feat=x1feat,
        wpack1=wpack1,
        wpack2=wpack2,
        att1_rep=np.tile(att1[None, :], (128, 1)).astype(np.float16),
        att2_rep=np.tile(att2[None, :], (128, 1)).astype(np.float16),
        bias1_rep=np.tile(bias1[None, :], (128, 1)).astype(np.float32),
        bias2_rep=np.tile(bias2[None, :], (128, 1)).astype(np.float32),
        padrow1=(-BIG * np.sign(att1)[None, :]).astype(np.float16),
        padrow2=(-BIG * np.sign(att2)[None, :]).astype(np.float16),
        wfcT=Wfc.T.astype(np.float16).copy(),
        bfc_rep=np.tile(bfc[None, :], (GPC, 1)).astype(np.float32),
    )
    in_maps = []
    for c in range(NCORES):
        m = dict(shared)
        m["ell0"] = _wrap16(ell[0][c])
        m["ell1"] = _wrap16(ell[1][c])
        m["sel"] = np.ascontiguousarray(sel[c])
        m["own_base"] = np.ascontiguousarray(own_base[c])
        in_maps.append(m)

    cfg = dict(groups=tuple(groups), ncols=(ncols[0], ncols[1]), n_slots=n_slots)
    return cfg, in_maps


def _cfg_key(cfg):
    return cfg["groups"]


# ============================================================= device build
def dma_gather_raw(gp, out_ap, in_ap, idxs_ap, num_idxs, elem_size, elem_step,
                   queue_num=0):
    """dma_gather without the elem_size%256 assert (64B elems work on HW)."""
    stride_bytes = elem_step * mybir.dt.size(in_ap.dtype)
    stride_bytes_256 = exact_div(stride_bytes, 256)
    _in_ap = gp.lower_ap_dma(in_ap, for_custom_bir_dma=True)
    _idxs_ap = gp.lower_ap(idxs_ap)
    _out_ap = gp.lower_ap(out_ap)
    return gp.add_instruction(
        mybir.InstDMAGatherAnt(
            name=gp.bass.get_next_instruction_name(),
            ins=[*_in_ap, _idxs_ap, gp.lower_val_access(gp.to_reg(num_idxs))],
            outs=[_out_ap],
            transpose=False,
            num_idxs=num_idxs,
            elem_size=elem_size,
            stride_bytes_256=stride_bytes_256,
            gen_mode=0,
            single_packet=False,
            queue_num=queue_num,
        )
    )


def _node_phase_conv1(nc, tc, ctx, x1feat, wp1_t, nodefeat, padrow1):
    A = 16
    with tc.tile_pool(name="n1sb", bufs=3) as sb, tc.tile_pool(
        name="n1ps", bufs=3, space="PSUM"
    ) as ps:
        for g0 in range(0, TT, A):
            x1t = sb.tile([3, A * 128], F32, tag="x1t")
            nc.gpsimd.memset(x1t[0:1, :], 1.0)
            nc.sync.dma_start(
                out=x1t[1:3, :], in_=x1feat[:, g0 * 128 : (g0 + A) * 128]
            )
            stage = sb.tile([128, A * 64], F16, tag="n1stage")
            for a4 in range(0, A, 4):
                pt = ps.tile([128, 4 * 64], F32, tag="n1psum")
                for a in range(4):
                    nc.tensor.matmul(
                        out=pt[:, a * 64 : (a + 1) * 64],
                        lhsT=x1t[:, (a4 + a) * 128 : (a4 + a + 1) * 128],
                        rhs=wp1_t[:],
                        start=True,
                        stop=True,
                    )
                nc.scalar.copy(out=stage[:, a4 * 64 : (a4 + 4) * 64], in_=pt[:])
            dst = bass.AP(
                nodefeat[:].tensor,
                g0 * 128,
                [[TC1 * 128, 128], [128, A], [1, 64]],
            )
            nc.sync.dma_start(out=dst, in_=stage[:])
        pr = sb.tile([1, HC], F16, tag="n1pad")
        nc.sync.dma_start(out=pr[:], in_=padrow1[:])
        nc.sync.dma_start(out=nodefeat[SENT : SENT + 1, 0:HC], in_=pr[:])


def _node_phase_conv2(nc, tc, ctx, agout, wp2_t, nodefeat, padrow2):
    A = 10
    with tc.tile_pool(name="n2sb", bufs=3) as sb, tc.tile_pool(
        name="n2ps", bufs=3, space="PSUM"
    ) as ps:
        for c8 in range(NCORES):
            for t0 in range(0, NT, A):
                lh = sb.tile([33, A * 128], F16, tag="n2lhs")
                src_ap = bass.AP(
                    agout[:].tensor,
                    c8 * 33 * NPC + t0 * 128,
                    [[NPC, 33], [1, A * 128]],
                )
                nc.sync.dma_start(out=lh[:], in_=src_ap)
                stage = sb.tile([128, A * 64], F16, tag="n2stage")
                for a5 in range(0, A, 5):
                    pt = ps.tile([128, 5 * 64], F32, tag="n2psum")
                    for a in range(5):
                        nc.tensor.matmul(
                            out=pt[:, a * 64 : (a + 1) * 64],
                            lhsT=lh[:, (a5 + a) * 128 : (a5 + a + 1) * 128],
                            rhs=wp2_t[:],
                            start=True,
                            stop=True,
                        )
                    nc.scalar.copy(out=stage[:, a5 * 64 : (a5 + 5) * 64],
                                   in_=pt[:])
                dst = bass.AP(
                    nodefeat[:].tensor,
                    (c8 * NT + t0) * 128,
                    [[TC1 * 128, 128], [128, A], [1, 64]],
                )
                nc.sync.dma_start(out=dst, in_=stage[:])
        pr = sb.tile([1, HC], F16, tag="n2pad")
        nc.sync.dma_start(out=pr[:], in_=padrow2[:])
        nc.sync.dma_start(out=nodefeat[SENT : SENT + 1, 0:HC], in_=pr[:])


def _edge_phase(nc, tc, ctx, cfg, conv, nodefeat, ell_d, att_t, bias_t, own_base_t,
                x2t_stage, x3_t, sel_t, pool_ps, identity_t):
    """One GATv2 conv aggregation over group-uniform ELL tiles.

    conv=1: writes transposed x2 into x2t_stage [33, NPC] (fp16).
    conv=2: writes x3 tiles into x3_t [128, NT*HC] (fp16) and accumulates the
            pooling matmul into pool_ps [HC, GPC].
    """
    groups = cfg["groups"]
    MAXH = max(max(T * k0, T * k1) for (_, T, k0, k1, _, _) in groups)
    MAXT = max(T for (_, T, k0, k1, _, _) in groups)
    sbg = ctx.enter_context(tc.tile_pool(name=f"e{conv}sbg", bufs=6))
    sbi = ctx.enter_context(tc.tile_pool(name=f"e{conv}sbi", bufs=6))
    sbz = ctx.enter_context(tc.tile_pool(name=f"e{conv}sbz", bufs=2))
    sbm = ctx.enter_context(tc.tile_pool(name=f"e{conv}sbm", bufs=3))
    sbs = ctx.enter_context(tc.tile_pool(name=f"e{conv}sbs", bufs=2))
    ps = ctx.enter_context(tc.tile_pool(name=f"e{conv}ps", bufs=2, space="PSUM"))

    # all own-node rows (xl|xr fp16, full 256B rows) for this core
    xrp = ctx.enter_context(tc.tile_pool(name=f"e{conv}xr", bufs=1))
    xrall = xrp.tile([128, NT * 128], F16, name=f"xrall{conv}")
    nc.gpsimd.indirect_dma_start(
        out=xrall[:],
        out_offset=None,
        in_=nodefeat[:],
        in_offset=bass.IndirectOffsetOnAxis(ap=own_base_t[:], axis=0),
        element_offset=0,
    )

    def ap4(t, off, dims):
        return bass.AP(t[:].tensor, t[:].offset + off, [t[:].ap[0]] + dims)

    pdim = xrall[:].ap[0]
    NGRP = len(groups)
    for gi in range(NGRP - 1, -1, -1):          # big groups first
        t0, T, gk0, gk1, off0, off1 = groups[gi]
        gk = (gk0, gk1)
        goff = (off0, off1)
        gbuf = [None, None]
        for h in range(2):
            nidx = 128 * T * gk[h]
            i0 = goff[h] * 128
            idx_t = sbi.tile([128, nidx // 16], I16, tag=f"idx{h}",
                             padded_shape=[128, MAXH * 8])
            nc.sync.dma_start(
                out=idx_t[:], in_=ell_d[h][:, i0 // 16 : (i0 + nidx) // 16]
            )
            gbuf[h] = sbg.tile(
                [128, T * gk[h] * HC], F16, tag=f"g{h}", name=f"gbuf{h}",
                padded_shape=[128, MAXH * HC],
            )
            base = W1LO if h == 1 else 0
            dma_gather_raw(
                nc.gpsimd,
                gbuf[h][:].rearrange("p (n e) -> p n e", e=HC),
                nodefeat[base : base + 32768, 0:HC],
                idx_t[:],
                nidx,
                HC,
                128,
                queue_num=0,
            )

        # per-half chain: z -> lrelu -> *att -> score -> exp -> msg (frees gbuf)
        ph = [None, None]
        den_h = [None, None]
        nh = [None, None]
        for h in range(2):
            TK = T * gk[h]
            z_t = sbz.tile([128, TK * HC], F16, tag="z",
                           padded_shape=[128, MAXH * HC])
            nc.vector.tensor_tensor(
                out=z_t[:],
                in0=gbuf[h][:],
                in1=bass.AP(xrall[:].tensor, xrall[:].offset + t0 * 128 + HC,
                            [pdim, [128, T], [0, gk[h]], [1, HC]]),
                op=ALU.add,
            )
            e_t = sbz.tile([128, TK * HC], F16, tag="e",
                           padded_shape=[128, MAXH * HC])
            nc.vector.scalar_tensor_tensor(
                out=e_t[:], in0=z_t[:], scalar=0.2, in1=z_t[:],
                op0=ALU.mult, op1=ALU.max,
            )
            ea_t = sbz.tile([128, TK * HC], F16, tag="ea",
                            padded_shape=[128, MAXH * HC])
            nc.vector.tensor_tensor(
                out=ea_t[:],
                in0=e_t[:],
                in1=bass.AP(att_t[:].tensor, att_t[:].offset,
                            [att_t[:].ap[0], [0, TK], [1, HC]]),
                op=ALU.mult,
            )
            s_t = sbs.tile([128, TK * H], F32, tag="s",
                           padded_shape=[128, MAXH * H])
            nc.vector.tensor_reduce(
                out=s_t[:],
                in_=ap4(ea_t, 0, [[HC, TK], [C, H], [1, C]]),
                axis=AX.X,
                op=ALU.add,
            )
            sc_t = sbs.tile([128, TK * H], F32, tag="sc",
                            padded_shape=[128, MAXH * H])
            nc.vector.tensor_scalar(
                out=sc_t[:], in0=s_t[:], scalar1=-80.0, scalar2=None,
                op0=ALU.max,
            )
            p_t = sbs.tile([128, TK * H], F32, tag="p", name=f"p{h}",
                           padded_shape=[128, MAXH * H])
            nc.scalar.activation(p_t[:], sc_t[:], ACTF.Exp)
            ph[h] = p_t
            den_h[h] = sbs.tile([128, T * H], F32, tag="den", name=f"den{h}",
                                padded_shape=[128, MAXT * H])
            nc.vector.tensor_reduce(
                out=den_h[h][:],
                in_=ap4(p_t, 0, [[gk[h] * H, T], [1, H], [H, gk[h]]]),
                axis=AX.X,
                op=ALU.add,
            )
        den_t = sbs.tile([128, T * H], F32, tag="dens",
                         padded_shape=[128, MAXT * H])
        nc.vector.tensor_add(out=den_t[:], in0=den_h[0][:], in1=den_h[1][:])
        rden_t = sbs.tile([128, T * H], F32, tag="rden",
                          padded_shape=[128, MAXT * H])
        nc.vector.reciprocal(out=rden_t[:], in_=den_t[:])
        # alpha = p*rden (fp16, per half) ; msg = xl*alpha ; num_h = sum_k msg
        for h in range(2):
            TK = T * gk[h]
            al_t = sbs.tile([128, TK * H], F16, tag="al", name=f"al{h}",
                            padded_shape=[128, MAXH * H])
            nc.vector.tensor_tensor(
                out=al_t[:],
                in0=ph[h][:],
                in1=ap4(rden_t, 0, [[H, T], [0, gk[h]], [1, H]]),
                op=ALU.mult,
            )
            msg = sbm.tile([128, TK * HC], F16, tag="m",
                           padded_shape=[128, MAXH * HC])
            nc.vector.tensor_tensor(
                out=msg[:],
                in0=gbuf[h][:],
                in1=ap4(al_t, 0, [[H, TK], [1, H], [0, C]]),
                op=ALU.mult,
            )
            nh[h] = sbs.tile([128, T * HC], F32, tag=f"n{h}", name=f"nh{h}",
                             padded_shape=[128, MAXT * HC])
            nc.vector.tensor_reduce(
                out=nh[h][:],
                in_=ap4(msg, 0, [[gk[h] * HC, T], [1, HC], [HC, gk[h]]]),
                axis=AX.X,
                op=ALU.add,
            )
        num_t = sbs.tile([128, T * HC], F32, tag="num",
                         padded_shape=[128, MAXT * HC])
        nc.vector.tensor_add(out=num_t[:], in0=nh[0][:], in1=nh[1][:])
        # o = num + bias ; x2 = relu(o) + exp(clamp(min(o,0),-80)) (+1 folded)
        o_t = sbs.tile([128, T * HC], F32, tag="o",
                       padded_shape=[128, MAXT * HC])
        nc.vector.tensor_tensor(
            out=o_t[:],
            in0=num_t[:],
            in1=bass.AP(bias_t[:].tensor, bias_t[:].offset,
                        [bias_t[:].ap[0], [0, T], [1, HC]]),
            op=ALU.add,
        )
        mn_t = sbs.tile([128, T * HC], F32, tag="mn",
                        padded_shape=[128, MAXT * HC])
        nc.vector.tensor_scalar(
            out=mn_t[:], in0=o_t[:], scalar1=0.0, scalar2=-80.0,
            op0=ALU.min, op1=ALU.max,
        )
        ex_t = sbs.tile([128, T * HC], F32, tag="ex",
                        padded_shape=[128, MAXT * HC])
        nc.scalar.activation(ex_t[:], mn_t[:], ACTF.Exp)
        if conv == 1:
            x2g = sbs.tile([128, T * HC], F16, tag="x2g",
                           padded_shape=[128, MAXT * HC])
            nc.vector.scalar_tensor_tensor(
                out=x2g[:], in0=o_t[:], scalar=0.0, in1=ex_t[:],
                op0=ALU.max, op1=ALU.add,
            )
            # transpose into x2t_stage [33, NPC] via PE, 4 tiles per transpose
            for ch0 in range(0, T, 4):
                cw = min(4, T - ch0) * HC
                tp = ps.tile([128, 128], F16, tag="tp")
                nc.tensor.transpose(
                    out=tp[0:cw, :],
                    in_=x2g[:, ch0 * HC : ch0 * HC + cw],
                    identity=identity_t[:],
                )
                for k in range((cw) // HC):
                    tt = t0 + ch0 + k
                    nc.scalar.copy(
                        out=x2t_stage[0:HC, tt * 128 : (tt + 1) * 128],
                        in_=tp[k * HC : (k + 1) * HC, :],
                    )
        else:
            nc.vector.scalar_tensor_tensor(
                out=x3_t[:, t0 * HC : (t0 + T) * HC], in0=o_t[:], scalar=0.0,
                in1=ex_t[:], op0=ALU.max, op1=ALU.add,
            )
            first_exec = groups[-1][0]
            last_exec = groups[0][0] + groups[0][1] - 1
            for k in range(T):
                tt = t0 + k
                nc.tensor.matmul(
                    out=pool_ps[:],
                    lhsT=x3_t[:, tt * HC : (tt + 1) * HC],
                    rhs=sel_t[:, tt * GPC : (tt + 1) * GPC],
                    start=(tt == first_exec),
                    stop=(tt == last_exec),
                )


def _build(cfg):
    nc = bacc.Bacc("TRN2", target_bir_lowering=False, debug=False,
                   num_devices=NCORES)
    ncol0, ncol1 = cfg["ncols"]

    x1feat = nc.dram_tensor("x1feat", [2, NG], F32, kind="ExternalInput").ap()
    ell0 = nc.dram_tensor("ell0", [128, ncol0 * 8], I16, kind="ExternalInput").ap()
    ell1 = nc.dram_tensor("ell1", [128, ncol1 * 8], I16, kind="ExternalInput").ap()
    sel = nc.dram_tensor("sel", [128, NT * GPC], F16, kind="ExternalInput").ap()
    own_base = nc.dram_tensor("own_base", [128, 1], I32, kind="ExternalInput").ap()
    wpack1 = nc.dram_tensor("wpack1", [3, 64], F32, kind="ExternalInput").ap()
    wpack2 = nc.dram_tensor("wpack2", [33, 64], F16, kind="ExternalInput").ap()
    att1_rep = nc.dram_tensor("att1_rep", [128, HC], F16, kind="ExternalInput").ap()
    att2_rep = nc.dram_tensor("att2_rep", [128, HC], F16, kind="ExternalInput").ap()
    bias1_rep = nc.dram_tensor("bias1_rep", [128, HC], F32, kind="ExternalInput").ap()
    bias2_rep = nc.dram_tensor("bias2_rep", [128, HC], F32, kind="ExternalInput").ap()
    padrow1 = nc.dram_tensor("padrow1", [1, HC], F16, kind="ExternalInput").ap()
    padrow2 = nc.dram_tensor("padrow2", [1, HC], F16, kind="ExternalInput").ap()
    wfcT = nc.dram_tensor("wfcT", [HC, 2], F16, kind="ExternalInput").ap()
    bfc_rep = nc.dram_tensor("bfc_rep", [GPC, 2], F32, kind="ExternalInput").ap()
    logits_out = nc.dram_tensor("logits", [GPC, 2], F32, kind="ExternalOutput").ap()

    with tile.TileContext(nc) as tc:
        from contextlib import ExitStack

        with ExitStack() as top:
            dram = top.enter_context(tc.tile_pool(name="dram", bufs=1, space="DRAM"))
            nodefeat1 = dram.tile([ROWS, 128], F16)
            nodefeat2 = dram.tile([ROWS, 128], F16)
            agin = dram.tile([33, NPC], F16)
            agout = dram.tile([NCORES * 33, NPC], F16)

            consts = top.enter_context(tc.tile_pool(name="consts", bufs=1))
            wp1_t = consts.tile([3, 64], F32)
            nc.sync.dma_start(out=wp1_t[:], in_=wpack1[:])
            wp2_t = consts.tile([33, 64], F16)
            nc.sync.dma_start(out=wp2_t[:], in_=wpack2[:])
            att1_t = consts.tile([128, HC], F16)
            nc.sync.dma_start(out=att1_t[:], in_=att1_rep[:])
            att2_t = consts.tile([128, HC], F16)
            nc.sync.dma_start(out=att2_t[:], in_=att2_rep[:])
            bias1_t = consts.tile([128, HC], F32)
            nc.sync.dma_start(out=bias1_t[:], in_=bias1_rep[:])
            bias2_t = consts.tile([128, HC], F32)
            nc.sync.dma_start(out=bias2_t[:], in_=bias2_rep[:])
            idf32 = consts.tile([128, 128], F32)
            make_identity(nc, idf32[:])
            identity_t = consts.tile([128, 128], F16)
            nc.vector.tensor_copy(out=identity_t[:], in_=idf32[:])
            own_base_t = consts.tile([128, 1], I32)
            nc.sync.dma_start(out=own_base_t[:], in_=own_base[:])
            sel_t = consts.tile([128, NT * GPC], F16)
            nc.sync.dma_start(out=sel_t[:], in_=sel[:])

            # ---------------- conv1
            _node_phase_conv1(nc, tc, top, x1feat, wp1_t, nodefeat1, padrow1)

            with ExitStack() as c1:
                stage_pool = c1.enter_context(tc.tile_pool(name="x2tst", bufs=1))
                x2t_stage = stage_pool.tile([33, NPC], F16)
                nc.gpsimd.memset(x2t_stage[32:33, :], 1.0)
                _edge_phase(nc, tc, c1, cfg, 1, nodefeat1, (ell0, ell1),
                            att1_t, bias1_t, own_base_t, x2t_stage, None, None,
                            None, identity_t)
                nc.sync.dma_start(out=agin[:], in_=x2t_stage[:])

            nc.gpsimd.collective_compute(
                "AllGather",
                ALU.bypass,
                replica_groups=[list(range(NCORES))],
                ins=[agin[:].opt()],
                outs=[agout[:].opt()],
            )

            # ---------------- conv2
            _node_phase_conv2(nc, tc, top, agout, wp2_t, nodefeat2, padrow2)

            with ExitStack() as c2:
                x3p = c2.enter_context(tc.tile_pool(name="x3p", bufs=1))
                x3_t = x3p.tile([128, NT * HC], F16)
                pps = c2.enter_context(tc.tile_pool(name="poolps", bufs=1,
                                                    space="PSUM"))
                pool_ps = pps.tile([HC, GPC], F32)
                _edge_phase(nc, tc, c2, cfg, 2, nodefeat2, (ell0, ell1),
                            att2_t, bias2_t, own_base_t, None, x3_t, sel_t,
                            pool_ps, identity_t)

                # ---------------- fc + log_softmax
                sb = c2.enter_context(tc.tile_pool(name="fcsb", bufs=1))
                ps2 = c2.enter_context(tc.tile_pool(name="fcps", bufs=1,
                                                    space="PSUM"))
                wfc_t = sb.tile([HC, 2], F16)
                nc.sync.dma_start(out=wfc_t[:], in_=wfcT[:])
                bfc_t = sb.tile([GPC, 2], F32)
                nc.sync.dma_start(out=bfc_t[:], in_=bfc_rep[:])
                pooledT_t = sb.tile([HC, GPC], F16)
                nc.scalar.copy(out=pooledT_t[:], in_=pool_ps[:])
                lg_ps = ps2.tile([GPC, 2], F32)
                nc.tensor.matmul(out=lg_ps[:], lhsT=pooledT_t[:],
                                 rhs=wfc_t[:], start=True, stop=True)
                lg_t = sb.tile([GPC, 2], F32)
                nc.vector.tensor_add(out=lg_t[:], in0=lg_ps[:], in1=bfc_t[:])
                mx_t = sb.tile([GPC, 1], F32)
                nc.vector.tensor_reduce(out=mx_t[:], in_=lg_t[:], axis=AX.X,
                                        op=ALU.max)
                sh_t = sb.tile([GPC, 2], F32)
                nc.vector.tensor_scalar(
                    out=sh_t[:], in0=lg_t[:], scalar1=mx_t[:, 0:1],
                    scalar2=None, op0=ALU.subtract,
                )
                exl_t = sb.tile([GPC, 2], F32)
                nc.scalar.activation(exl_t[:], sh_t[:], ACTF.Exp)
                se_t = sb.tile([GPC, 1], F32)
                nc.vector.tensor_reduce(out=se_t[:], in_=exl_t[:], axis=AX.X,
                                        op=ALU.add)
                ln_t = sb.tile([GPC, 1], F32)
                nc.scalar.activation(ln_t[:], se_t[:], ACTF.Ln)
                out_t = sb.tile([GPC, 2], F32)
                nc.vector.tensor_scalar(
                    out=out_t[:], in0=sh_t[:], scalar1=ln_t[:, 0:1],
                    scalar2=None, op0=ALU.subtract,
                )
                nc.sync.dma_start(out=logits_out[:], in_=out_t[:])

    nc.compile()
    return nc


# =================================================================== driver
_CACHE = {}


def kernel(**inputs) -> np.ndarray:
    cfg, in_maps = _prep(inputs)
    key = _cfg_key(cfg)
    if key not in _CACHE:
        _CACHE[key] = _build(cfg)
    nc = _CACHE[key]
    res = bass_utils.run_bass_kernel_spmd(nc, in_maps, core_ids=list(range(NCORES)))
    out = np.concatenate([res.results[c]["logits"] for c in range(NCORES)], axis=0)
    return out.astype(np.float32)


# revision 21
# speedup vs baseline: 1.2864x; 1.0059x over previous
"""GATv2 graph net (IMDB) Trainium2 kernel — 8-core SPMD, fp16 edge phase.

Architecture (v2):
- dst-partition edges across 8 cores on graph-aligned node ranges; per-core
  degree-sorted padded ELL (rows = destination nodes, slots on the free dim).
- Node features (xl|xr) are stored fp16 in a replicated DRAM table with 256B
  rows addressed by a transposed tiling r = (g%128)*401 + g//128.  Per-slot
  source xl (64B) comes in via SWDGE dma_gather.  int16 gather indices only
  span 32768 rows, so two overlapping windows are used: [0,32768) and
  [18560,51328).  Sources whose row falls in the overlap are assigned per
  destination row to whichever window balances the two slot counts, which
  keeps per-tile ELL padding low (~1.3x vs 1.7x for a blind split).
- Edge compute runs on group-uniform slot counts: G consecutive tiles share
  one K per window so the whole group is processed by ~20 large fp16 vector
  instructions (4D access patterns, 2x DVE mode) instead of ~26 per tile.
  exp() stays f32 (fp16 would overflow); alpha is normalized before the
  weighted message sum so everything downstream of exp is fp16 again.
- ELU's -1 is folded into the next layer's bias (b2' = b2 - W2@1,
  bfc' = bfc - Wfc@1), so the edge phase emits relu(o)+exp(min(o,0)).
- Mean-pool is a matmul: pooled^T = sum_t x3_tile^T @ Sel_tile where Sel has
  1/graph_size at (node row, local graph) — no gather, no transposes, and the
  fc layer consumes pooled^T directly.
- Between convs the per-core x2^T is AllGathered in fp16.
"""

import sys

sys.path.insert(0, "/opt/trn_rl_repo")

import numpy as np

import concourse.bass as bass
import concourse.bacc as bacc
import concourse.tile as tile
from concourse import mybir
from concourse.bass import exact_div
from concourse.masks import make_identity
from concourse import bass_utils

# ---------------------------------------------------------------- constants
N = 50000
E = 1_600_000
H = 2
C = 16
HC = H * C            # 32
G = 512
NCORES = 8
GPC = G // NCORES     # graphs per core = 64
NPC = 6400            # padded nodes per core (50 tiles of 128)
NT = NPC // 128       # node tiles per core = 50
NG = NCORES * NPC     # padded global node count = 51200
TT = NG // 128        # 400 tiles in gid space
TC1 = TT + 1          # row-columns per partition (400 tiles + 1 spare)
ROWS = 128 * TC1      # node table rows = 51328
W1LO = ROWS - 32768   # window1 base row = 18560
SENT = 60 * TC1 + 400 # sentinel row (spare column, inside the overlap) = 24460+400
BIG = 49152.0         # fp16-safe saturation value for sentinel xl
BUD = 64              # group budget: T*(K0+K1) <= BUD
TMAX = 12             # max tiles per group
F32 = mybir.dt.float32
F16 = mybir.dt.float16
I32 = mybir.dt.int32
I16 = mybir.dt.int16
AX = mybir.AxisListType
ALU = mybir.AluOpType
ACTF = mybir.ActivationFunctionType


def _r_of_gid(g):
    """node-table row for gid: transposed tiling so per-partition rows of one
    core are consecutive (xr indirect read) and node-phase writes batch."""
    return (g % 128) * TC1 + g // 128


def _wrap16(flat):
    """dma_gather index layout: flat i -> [16*g + i%16, i//16], replicated
    across the 8 Q7-core partition groups."""
    flat = np.asarray(flat, np.int16)
    n = len(flat)
    assert n % 16 == 0
    arr = np.empty((128, n // 16), np.int16)
    blk = flat.reshape(n // 16, 16).T
    for g in range(8):
        arr[g * 16 : (g + 1) * 16, :] = blk
    return arr


# ================================================================ host prep
def _prep(inputs):
    ei = np.asarray(inputs["edge_index"])
    src = ei[0].astype(np.int64)
    dst = ei[1].astype(np.int64)
    batch = np.asarray(inputs["batch"]).astype(np.int64)
    rand_feat = np.asarray(inputs["rand_feat"], dtype=np.float32).reshape(-1)

    deg = (np.bincount(src, minlength=N) + np.bincount(dst, minlength=N)).astype(
        np.float32
    )
    ddeg = (np.bincount(dst, minlength=N) + 1).astype(np.int64)  # + self loop

    # graph-aligned core boundaries
    bounds = np.searchsorted(batch, np.arange(0, G + 1, GPC))
    assert bounds[0] == 0 and bounds[-1] == N
    ncs = np.diff(bounds)
    assert ncs.max() <= NPC, ncs.max()

    # per-core degree-sorted node order; gid = core*NPC + rank
    gid = np.empty(N, np.int64)
    for c in range(NCORES):
        lo, hi = bounds[c], bounds[c + 1]
        order = np.argsort(ddeg[lo:hi], kind="stable") + lo
        gid[order] = c * NPC + np.arange(hi - lo)

    # edges (+self loops)
    src_sl = np.concatenate([src, np.arange(N, dtype=np.int64)])
    dst_sl = np.concatenate([dst, np.arange(N, dtype=np.int64)])
    EP = len(src_sl)
    sr = _r_of_gid(gid[src_sl])          # source node-table row
    dgid = gid[dst_sl]

    # ---- window assignment with per-dst balancing over the overlap band
    forced1 = sr >= 32768
    forced0 = sr < W1LO
    flexm = (~forced0) & (~forced1)
    f0 = np.bincount(dgid[forced0], minlength=NG)
    f1 = np.bincount(dgid[forced1], minlength=NG)
    fx = np.bincount(dgid[flexm], minlength=NG)
    degg = f0 + f1 + fx
    quota0 = np.clip((f1 + fx - f0 + 1) // 2, 0, fx)  # flex slots -> window0
    fill0 = f0 + quota0
    fill1 = degg - fill0

    # order edges by dst gid; rank flex edges within each dst
    eorder = np.argsort(dgid, kind="stable")
    dg_s = dgid[eorder]
    sr_s = sr[eorder]
    flex_s = flexm[eorder]
    half_s = forced1[eorder].astype(np.int64)
    fidx = np.nonzero(flex_s)[0]
    dgf = dg_s[fidx]                      # sorted (dg_s sorted)
    frank = np.arange(len(fidx)) - np.searchsorted(dgf, dgf, side="left")
    half_s[fidx] = (frank >= quota0[dgf]).astype(np.int64)

    # slot index within (dst, half); slots ordered by source row so the
    # gather's descriptors walk ascending HBM addresses (row-buffer locality)
    key2 = dg_s * 2 + half_s
    k2o = np.lexsort((sr_s, key2))
    ks_start = np.searchsorted(key2[k2o], np.arange(NG * 2))
    slot = np.empty(EP, np.int64)
    slot[k2o] = np.arange(EP) - ks_start[key2[k2o]]

    # per-tile K per window (max over cores and rows)
    F0 = fill0.reshape(NCORES, NT, 128)
    F1 = fill1.reshape(NCORES, NT, 128)
    K0t = F0.max(axis=(0, 2)).astype(np.int64)
    K1t = F1.max(axis=(0, 2)).astype(np.int64)

    # greedy grouping: T consecutive tiles share (K0, K1); T*(K0+K1) <= BUD
    groups = []
    t = 0
    while t < NT:
        T = 1
        k0 = int(K0t[t]); k1 = int(K1t[t])
        while t + T < NT and T < TMAX:
            nk0 = max(k0, int(K0t[t + T])); nk1 = max(k1, int(K1t[t + T]))
            if (T + 1) * (nk0 + nk1) > BUD:
                break
            T += 1; k0, k1 = nk0, nk1
        groups.append((t, T, k0, k1))
        t += T

    colbase = np.zeros((2, NT), np.int64)   # slot-column start per (half, tile)
    goff = [0, 0]
    groups2 = []
    for (t0, T, k0, k1) in groups:
        assert k0 >= 1 and k1 >= 1
        groups2.append((t0, T, k0, k1, goff[0], goff[1]))
        for tt in range(T):
            colbase[0, t0 + tt] = goff[0] + tt * k0
            colbase[1, t0 + tt] = goff[1] + tt * k1
        goff[0] += T * k0
        goff[1] += T * k1
    groups = groups2
    ncols = [int(goff[0]), int(goff[1])]
    n_slots = (ncols[0] + ncols[1]) * 128

    # ---- ELL index arrays (wrapped int16), pads -> sentinel
    core_of = dg_s // NPC
    j = dg_s % NPC
    tt_ = j // 128
    p_ = j % 128
    ell = []
    for h in range(2):
        base = W1LO if h == 1 else 0
        sent_rel = SENT - base
        flat = np.full((NCORES, ncols[h] * 128), sent_rel, np.int64)
        selm = np.nonzero(half_s == h)[0]
        ghk = [(g[2] if h == 0 else g[3]) for g in groups]
        # group K per tile for assert
        pos = (colbase[h][tt_[selm]] + slot[selm]) * 128 + p_[selm]
        flat[core_of[selm], pos] = sr_s[selm] - base
        assert flat.max() <= 32767 and flat.min() >= 0
        ell.append(flat)

    # ---- pooling selection matrix (1/graph_size at (row, local graph))
    gsz = np.bincount(batch, minlength=G).astype(np.float64)
    sel = np.zeros((NCORES, 128, NT * GPC), np.float16)
    # node n sits at gid[n] = c*NPC + j ; local graph = batch[n] - c*GPC
    nc_ = gid // NPC
    nj = gid % NPC
    npp = nj % 128
    ntt = nj // 128
    lg = batch - nc_ * GPC
    sel[nc_, npp, ntt * GPC + lg] = (1.0 / np.maximum(gsz[batch], 1.0)).astype(
        np.float16
    )

    # ---- x1 features in gid-column order [2, NG]
    x1feat = np.zeros((2, NG), np.float32)
    x1feat[0, gid] = deg
    x1feat[1, gid] = rand_feat

    # ---- own-row base for the per-conv xr indirect read
    own_base = np.empty((NCORES, 128, 1), np.int32)
    for c in range(NCORES):
        own_base[c, :, 0] = np.arange(128) * TC1 + c * NT

    # ---- packed weights
    def f32(x):
        return np.asarray(x, dtype=np.float32)

    W1l, W1r = f32(inputs["W1l"]), f32(inputs["W1r"])
    b1 = np.concatenate([f32(inputs["b1l"]), f32(inputs["b1r"])])
    W2l, W2r = f32(inputs["W2l"]), f32(inputs["W2r"])
    # fold ELU's -1 (x2_stored = x2_true + 1) into conv2 bias
    b2 = np.concatenate(
        [f32(inputs["b2l"]) - W2l.sum(axis=1), f32(inputs["b2r"]) - W2r.sum(axis=1)]
    )
    att1 = f32(inputs["att1"]).reshape(HC)
    att2 = f32(inputs["att2"]).reshape(HC)
    bias1 = f32(inputs["bias1"])
    bias2 = f32(inputs["bias2"])
    Wfc, bfc = f32(inputs["Wfc"]), f32(inputs["bfc"])
    bfc = bfc - Wfc.sum(axis=1)  # fold x3_stored = x3_true + 1

    wpack1 = np.concatenate([W1l.T, W1r.T], axis=1)
    wpack1[0, :] += b1
    wpack2 = np.concatenate([W2l.T, W2r.T], axis=1)
    wpack2 = np.concatenate([wpack2, b2[None, :]], axis=0).astype(np.float16)

    shared = dict(
        x1feat=x1feat,
        wpack1=wpack1,
        wpack2=wpack2,
        att1_rep=np.tile(att1[None, :], (128, 1)).astype(np.float16),
        att2_rep=np.tile(att2[None, :], (128, 1)).astype(np.float16),
        bias1_rep=np.tile(bias1[None, :], (128, 1)).astype(np.float32),
        bias2_rep=np.tile(bias2[None, :], (128, 1)).astype(np.float32),
        padrow1=(-BIG * np.sign(att1)[None, :]).astype(np.float16),
        padrow2=(-BIG * np.sign(att2)[None, :]).astype(np.float16),
        wfcT=Wfc.T.astype(np.float16).copy(),
        bfc_rep=np.tile(bfc[None, :], (GPC, 1)).astype(np.float32),
    )
    in_maps = []
    for c in range(NCORES):
        m = dict(shared)
        m["ell0"] = _wrap16(ell[0][c])
        m["ell1"] = _wrap16(ell[1][c])
        m["sel"] = np.ascontiguousarray(sel[c])
        m["own_base"] = np.ascontiguousarray(own_base[c])
        in_maps.append(m)

    cfg = dict(groups=tuple(groups), ncols=(ncols[0], ncols[1]), n_slots=n_slots)
    return cfg, in_maps


def _cfg_key(cfg):
    return cfg["groups"]


# ============================================================= device build
def dma_gather_raw(gp, out_ap, in_ap, idxs_ap, num_idxs, elem_size, elem_step,
                   queue_num=0):
    """dma_gather without the elem_size%256 assert (64B elems work on HW)."""
    stride_bytes = elem_step * mybir.dt.size(in_ap.dtype)
    stride_bytes_256 = exact_div(stride_bytes, 256)
    _in_ap = gp.lower_ap_dma(in_ap, for_custom_bir_dma=True)
    _idxs_ap = gp.lower_ap(idxs_ap)
    _out_ap = gp.lower_ap(out_ap)
    return gp.add_instruction(
        mybir.InstDMAGatherAnt(
            name=gp.bass.get_next_instruction_name(),
            ins=[*_in_ap, _idxs_ap, gp.lower_val_access(gp.to_reg(num_idxs))],
            outs=[_out_ap],
            transpose=False,
            num_idxs=num_idxs,
            elem_size=elem_size,
            stride_bytes_256=stride_bytes_256,
            gen_mode=0,
            single_packet=False,
            queue_num=queue_num,
        )
    )


def _node_phase_conv1(nc, tc, ctx, x1feat, wp1_t, nodefeat, padrow1):
    A = 16
    with tc.tile_pool(name="n1sb", bufs=3) as sb, tc.tile_pool(
        name="n1ps", bufs=3, space="PSUM"
    ) as ps:
        for g0 in range(0, TT, A):
            x1t = sb.tile([3, A * 128], F32, tag="x1t")
            nc.gpsimd.memset(x1t[0:1, :], 1.0)
            nc.sync.dma_start(
                out=x1t[1:3, :], in_=x1feat[:, g0 * 128 : (g0 + A) * 128]
            )
            stage = sb.tile([128, A * 64], F16, tag="n1stage")
            for a4 in range(0, A, 4):
                pt = ps.tile([128, 4 * 64], F32, tag="n1psum")
                for a in range(4):
                    nc.tensor.matmul(
                        out=pt[:, a * 64 : (a + 1) * 64],
                        lhsT=x1t[:, (a4 + a) * 128 : (a4 + a + 1) * 128],
                        rhs=wp1_t[:],
                        start=True,
                        stop=True,
                    )
                nc.scalar.copy(out=stage[:, a4 * 64 : (a4 + 4) * 64], in_=pt[:])
            dst = bass.AP(
                nodefeat[:].tensor,
                g0 * 128,
                [[TC1 * 128, 128], [128, A], [1, 64]],
            )
            nc.sync.dma_start(out=dst, in_=stage[:])
        pr = sb.tile([1, HC], F16, tag="n1pad")
        nc.sync.dma_start(out=pr[:], in_=padrow1[:])
        nc.sync.dma_start(out=nodefeat[SENT : SENT + 1, 0:HC], in_=pr[:])


def _node_phase_conv2(nc, tc, ctx, agout, wp2_t, nodefeat, padrow2):
    A = 10
    with tc.tile_pool(name="n2sb", bufs=3) as sb, tc.tile_pool(
        name="n2ps", bufs=3, space="PSUM"
    ) as ps:
        for c8 in range(NCORES):
            for t0 in range(0, NT, A):
                lh = sb.tile([33, A * 128], F16, tag="n2lhs")
                src_ap = bass.AP(
                    agout[:].tensor,
                    c8 * 33 * NPC + t0 * 128,
                    [[NPC, 33], [1, A * 128]],
                )
                nc.sync.dma_start(out=lh[:], in_=src_ap)
                stage = sb.tile([128, A * 64], F16, tag="n2stage")
                for a5 in range(0, A, 5):
                    pt = ps.tile([128, 5 * 64], F32, tag="n2psum")
                    for a in range(5):
                        nc.tensor.matmul(
                            out=pt[:, a * 64 : (a + 1) * 64],
                            lhsT=lh[:, (a5 + a) * 128 : (a5 + a + 1) * 128],
                            rhs=wp2_t[:],
                            start=True,
                            stop=True,
                        )
                    nc.scalar.copy(out=stage[:, a5 * 64 : (a5 + 5) * 64],
                                   in_=pt[:])
                dst = bass.AP(
                    nodefeat[:].tensor,
                    (c8 * NT + t0) * 128,
                    [[TC1 * 128, 128], [128, A], [1, 64]],
                )
                nc.sync.dma_start(out=dst, in_=stage[:])
        pr = sb.tile([1, HC], F16, tag="n2pad")
        nc.sync.dma_start(out=pr[:], in_=padrow2[:])
        nc.sync.dma_start(out=nodefeat[SENT : SENT + 1, 0:HC], in_=pr[:])


def _edge_phase(nc, tc, ctx, cfg, conv, nodefeat, ell_d, att_t, bias_t, own_base_t,
                x2t_stage, x3_t, sel_t, pool_ps, identity_t):
    """One GATv2 conv aggregation over group-uniform ELL tiles.

    conv=1: writes transposed x2 into x2t_stage [33, NPC] (fp16).
    conv=2: writes x3 tiles into x3_t [128, NT*HC] (fp16) and accumulates the
            pooling matmul into pool_ps [HC, GPC].
    """
    groups = cfg["groups"]
    MAXH = max(max(T * k0, T * k1) for (_, T, k0, k1, _, _) in groups)
    MAXT = max(T for (_, T, k0, k1, _, _) in groups)
    sbg = ctx.enter_context(tc.tile_pool(name=f"e{conv}sbg", bufs=6))
    sbi = ctx.enter_context(tc.tile_pool(name=f"e{conv}sbi", bufs=6))
    sbz = ctx.enter_context(tc.tile_pool(name=f"e{conv}sbz", bufs=2))
    sbm = ctx.enter_context(tc.tile_pool(name=f"e{conv}sbm", bufs=3))
    sbs = ctx.enter_context(tc.tile_pool(name=f"e{conv}sbs", bufs=2))
    ps = ctx.enter_context(tc.tile_pool(name=f"e{conv}ps", bufs=2, space="PSUM"))

    # all own-node rows (xl|xr fp16, full 256B rows) for this core
    xrp = ctx.enter_context(tc.tile_pool(name=f"e{conv}xr", bufs=1))
    xrall = xrp.tile([128, NT * 128], F16, name=f"xrall{conv}")
    nc.gpsimd.indirect_dma_start(
        out=xrall[:],
        out_offset=None,
        in_=nodefeat[:],
        in_offset=bass.IndirectOffsetOnAxis(ap=own_base_t[:], axis=0),
        element_offset=0,
    )

    def ap4(t, off, dims):
        return bass.AP(t[:].tensor, t[:].offset + off, [t[:].ap[0]] + dims)

    pdim = xrall[:].ap[0]
    NGRP = len(groups)
    for gi in range(NGRP - 1, -1, -1):          # big groups first
        t0, T, gk0, gk1, off0, off1 = groups[gi]
        gk = (gk0, gk1)
        goff = (off0, off1)
        gbuf = [None, None]
        for h in range(2):
            nidx = 128 * T * gk[h]
            i0 = goff[h] * 128
            idx_t = sbi.tile([128, nidx // 16], I16, tag=f"idx{h}",
                             padded_shape=[128, MAXH * 8])
            nc.sync.dma_start(
                out=idx_t[:], in_=ell_d[h][:, i0 // 16 : (i0 + nidx) // 16]
            )
            gbuf[h] = sbg.tile(
                [128, T * gk[h] * HC], F16, tag=f"g{h}", name=f"gbuf{h}",
                padded_shape=[128, MAXH * HC],
            )
            base = W1LO if h == 1 else 0
            dma_gather_raw(
                nc.gpsimd,
                gbuf[h][:].rearrange("p (n e) -> p n e", e=HC),
                nodefeat[base : base + 32768, 0:HC],
                idx_t[:],
                nidx,
                HC,
                128,
                queue_num=0,
            )

        # per-half chain: z -> lrelu -> *att -> score -> exp -> msg (frees gbuf)
        ph = [None, None]
        den_h = [None, None]
        nh = [None, None]
        for h in range(2):
            TK = T * gk[h]
            z_t = sbz.tile([128, TK * HC], F16, tag="z",
                           padded_shape=[128, MAXH * HC])
            nc.vector.tensor_tensor(
                out=z_t[:],
                in0=gbuf[h][:],
                in1=bass.AP(xrall[:].tensor, xrall[:].offset + t0 * 128 + HC,
                            [pdim, [128, T], [0, gk[h]], [1, HC]]),
                op=ALU.add,
            )
            e_t = sbz.tile([128, TK * HC], F16, tag="e",
                           padded_shape=[128, MAXH * HC])
            nc.vector.scalar_tensor_tensor(
                out=e_t[:], in0=z_t[:], scalar=0.2, in1=z_t[:],
                op0=ALU.mult, op1=ALU.max,
            )
            ea_t = sbz.tile([128, TK * HC], F16, tag="ea",
                            padded_shape=[128, MAXH * HC])
            nc.vector.tensor_tensor(
                out=ea_t[:],
                in0=e_t[:],
                in1=bass.AP(att_t[:].tensor, att_t[:].offset,
                            [att_t[:].ap[0], [0, TK], [1, HC]]),
                op=ALU.mult,
            )
            s_t = sbs.tile([128, TK * H], F32, tag="s",
                           padded_shape=[128, MAXH * H])
            nc.vector.tensor_reduce(
                out=s_t[:],
                in_=ap4(ea_t, 0, [[HC, TK], [C, H], [1, C]]),
                axis=AX.X,
                op=ALU.add,
            )
            sc_t = sbs.tile([128, TK * H], F32, tag="sc",
                            padded_shape=[128, MAXH * H])
            nc.vector.tensor_scalar(
                out=sc_t[:], in0=s_t[:], scalar1=-80.0, scalar2=None,
                op0=ALU.max,
            )
            p_t = sbs.tile([128, TK * H], F32, tag="p", name=f"p{h}",
                           padded_shape=[128, MAXH * H])
            nc.scalar.activation(p_t[:], sc_t[:], ACTF.Exp)
            ph[h] = p_t
            den_h[h] = sbs.tile([128, T * H], F32, tag="den", name=f"den{h}",
                                padded_shape=[128, MAXT * H])
            nc.vector.tensor_reduce(
                out=den_h[h][:],
                in_=ap4(p_t, 0, [[gk[h] * H, T], [1, H], [H, gk[h]]]),
                axis=AX.X,
                op=ALU.add,
            )
        den_t = sbs.tile([128, T * H], F32, tag="dens",
                         padded_shape=[128, MAXT * H])
        nc.vector.tensor_add(out=den_t[:], in0=den_h[0][:], in1=den_h[1][:])
        rden_t = sbs.tile([128, T * H], F32, tag="rden",
                          padded_shape=[128, MAXT * H])
        nc.vector.reciprocal(out=rden_t[:], in_=den_t[:])
        # alpha = p*rden (fp16, per half) ; msg = xl*alpha ; num_h = sum_k msg
        for h in range(2):
            TK = T * gk[h]
            al_t = sbs.tile([128, TK * H], F16, tag="al", name=f"al{h}",
                            padded_shape=[128, MAXH * H])
            nc.vector.tensor_tensor(
                out=al_t[:],
                in0=ph[h][:],
                in1=ap4(rden_t, 0, [[H, T], [0, gk[h]], [1, H]]),
                op=ALU.mult,
            )
            msg = sbm.tile([128, TK * HC], F16, tag="m",
                           padded_shape=[128, MAXH * HC])
            nc.vector.tensor_tensor(
                out=msg[:],
                in0=gbuf[h][:],
                in1=ap4(al_t, 0, [[H, TK], [1, H], [0, C]]),
                op=ALU.mult,
            )
            nh[h] = sbs.tile([128, T * HC], F32, tag=f"n{h}", name=f"nh{h}",
                             padded_shape=[128, MAXT * HC])
            nc.vector.tensor_reduce(
                out=nh[h][:],
                in_=ap4(msg, 0, [[gk[h] * HC, T], [1, HC], [HC, gk[h]]]),
                axis=AX.X,
                op=ALU.add,
            )
        num_t = sbs.tile([128, T * HC], F32, tag="num",
                         padded_shape=[128, MAXT * HC])
        nc.vector.tensor_add(out=num_t[:], in0=nh[0][:], in1=nh[1][:])
        # o = num + bias ; x2 = relu(o) + exp(clamp(min(o,0),-80)) (+1 folded)
        o_t = sbs.tile([128, T * HC], F32, tag="o",
                       padded_shape=[128, MAXT * HC])
        nc.vector.tensor_tensor(
            out=o_t[:],
            in0=num_t[:],
            in1=bass.AP(bias_t[:].tensor, bias_t[:].offset,
                        [bias_t[:].ap[0], [0, T], [1, HC]]),
            op=ALU.add,
        )
        mn_t = sbs.tile([128, T * HC], F32, tag="mn",
                        padded_shape=[128, MAXT * HC])
        nc.vector.tensor_scalar(
            out=mn_t[:], in0=o_t[:], scalar1=0.0, scalar2=-80.0,
            op0=ALU.min, op1=ALU.max,
        )
        ex_t = sbs.tile([128, T * HC], F32, tag="ex",
                        padded_shape=[128, MAXT * HC])
        nc.scalar.activation(ex_t[:], mn_t[:], ACTF.Exp)
        if conv == 1:
            x2g = sbs.tile([128, T * HC], F16, tag="x2g",
                           padded_shape=[128, MAXT * HC])
            nc.vector.scalar_tensor_tensor(
                out=x2g[:], in0=o_t[:], scalar=0.0, in1=ex_t[:],
                op0=ALU.max, op1=ALU.add,
            )
            # transpose into x2t_stage [33, NPC] via PE, 4 tiles per transpose
            for ch0 in range(0, T, 4):
                cw = min(4, T - ch0) * HC
                tp = ps.tile([128, 128], F16, tag="tp")
                nc.tensor.transpose(
                    out=tp[0:cw, :],
                    in_=x2g[:, ch0 * HC : ch0 * HC + cw],
                    identity=identity_t[:],
                )
                for k in range((cw) // HC):
                    tt = t0 + ch0 + k
                    nc.scalar.copy(
                        out=x2t_stage[0:HC, tt * 128 : (tt + 1) * 128],
                        in_=tp[k * HC : (k + 1) * HC, :],
                    )
        else:
            nc.vector.scalar_tensor_tensor(
                out=x3_t[:, t0 * HC : (t0 + T) * HC], in0=o_t[:], scalar=0.0,
                in1=ex_t[:], op0=ALU.max, op1=ALU.add,
            )
            first_exec = groups[-1][0]
            last_exec = groups[0][0] + groups[0][1] - 1
            for k in range(T):
                tt = t0 + k
                nc.tensor.matmul(
                    out=pool_ps[:],
                    lhsT=x3_t[:, tt * HC : (tt + 1) * HC],
                    rhs=sel_t[:, tt * GPC : (tt + 1) * GPC],
                    start=(tt == first_exec),
                    stop=(tt == last_exec),
                )


def _build(cfg):
    nc = bacc.Bacc("TRN2", target_bir_lowering=False, debug=False,
                   num_devices=NCORES)
    ncol0, ncol1 = cfg["ncols"]

    x1feat = nc.dram_tensor("x1feat", [2, NG], F32, kind="ExternalInput").ap()
    ell0 = nc.dram_tensor("ell0", [128, ncol0 * 8], I16, kind="ExternalInput").ap()
    ell1 = nc.dram_tensor("ell1", [128, ncol1 * 8], I16, kind="ExternalInput").ap()
    sel = nc.dram_tensor("sel", [128, NT * GPC], F16, kind="ExternalInput").ap()
    own_base = nc.dram_tensor("own_base", [128, 1], I32, kind="ExternalInput").ap()
    wpack1 = nc.dram_tensor("wpack1", [3, 64], F32, kind="ExternalInput").ap()
    wpack2 = nc.dram_tensor("wpack2", [33, 64], F16, kind="ExternalInput").ap()
    att1_rep = nc.dram_tensor("att1_rep", [128, HC], F16, kind="ExternalInput").ap()
    att2_rep = nc.dram_tensor("att2_rep", [128, HC], F16, kind="ExternalInput").ap()
    bias1_rep = nc.dram_tensor("bias1_rep", [128, HC], F32, kind="ExternalInput").ap()
    bias2_rep = nc.dram_tensor("bias2_rep", [128, HC], F32, kind="ExternalInput").ap()
    padrow1 = nc.dram_tensor("padrow1", [1, HC], F16, kind="ExternalInput").ap()
    padrow2 = nc.dram_tensor("padrow2", [1, HC], F16, kind="ExternalInput").ap()
    wfcT = nc.dram_tensor("wfcT", [HC, 2], F16, kind="ExternalInput").ap()
    bfc_rep = nc.dram_tensor("bfc_rep", [GPC, 2], F32, kind="ExternalInput").ap()
    logits_out = nc.dram_tensor("logits", [GPC, 2], F32, kind="ExternalOutput").ap()

    with tile.TileContext(nc) as tc:
        from contextlib import ExitStack

        with ExitStack() as top:
            dram = top.enter_context(tc.tile_pool(name="dram", bufs=1, space="DRAM"))
            nodefeat1 = dram.tile([ROWS, 128], F16)
            nodefeat2 = dram.tile([ROWS, 128], F16)
            agin = dram.tile([33, NPC], F16)
            agout = dram.tile([NCORES * 33, NPC], F16)

            consts = top.enter_context(tc.tile_pool(name="consts", bufs=1))
            wp1_t = consts.tile([3, 64], F32)
            nc.sync.dma_start(out=wp1_t[:], in_=wpack1[:])
            wp2_t = consts.tile([33, 64], F16)
            nc.sync.dma_start(out=wp2_t[:], in_=wpack2[:])
            att1_t = consts.tile([128, HC], F16)
            nc.sync.dma_start(out=att1_t[:], in_=att1_rep[:])
            att2_t = consts.tile([128, HC], F16)
            nc.sync.dma_start(out=att2_t[:], in_=att2_rep[:])
            bias1_t = consts.tile([128, HC], F32)
            nc.sync.dma_start(out=bias1_t[:], in_=bias1_rep[:])
            bias2_t = consts.tile([128, HC], F32)
            nc.sync.dma_start(out=bias2_t[:], in_=bias2_rep[:])
            idf32 = consts.tile([128, 128], F32)
            make_identity(nc, idf32[:])
            identity_t = consts.tile([128, 128], F16)
            nc.vector.tensor_copy(out=identity_t[:], in_=idf32[:])
            own_base_t = consts.tile([128, 1], I32)
            nc.sync.dma_start(out=own_base_t[:], in_=own_base[:])
            sel_t = consts.tile([128, NT * GPC], F16)
            nc.sync.dma_start(out=sel_t[:], in_=sel[:])

            # ---------------- conv1
            _node_phase_conv1(nc, tc, top, x1feat, wp1_t, nodefeat1, padrow1)

            with ExitStack() as c1:
                stage_pool = c1.enter_context(tc.tile_pool(name="x2tst", bufs=1))
                x2t_stage = stage_pool.tile([33, NPC], F16)
                nc.gpsimd.memset(x2t_stage[32:33, :], 1.0)
                _edge_phase(nc, tc, c1, cfg, 1, nodefeat1, (ell0, ell1),
                            att1_t, bias1_t, own_base_t, x2t_stage, None, None,
                            None, identity_t)
                nc.sync.dma_start(out=agin[:], in_=x2t_stage[:])

            nc.gpsimd.collective_compute(
                "AllGather",
                ALU.bypass,
                replica_groups=[list(range(NCORES))],
                ins=[agin[:].opt()],
                outs=[agout[:].opt()],
            )

            # ---------------- conv2
            _node_phase_conv2(nc, tc, top, agout, wp2_t, nodefeat2, padrow2)

            with ExitStack() as c2:
                x3p = c2.enter_context(tc.tile_pool(name="x3p", bufs=1))
                x3_t = x3p.tile([128, NT * HC], F16)
                pps = c2.enter_context(tc.tile_pool(name="poolps", bufs=1,
                                                    space="PSUM"))
                pool_ps = pps.tile([HC, GPC], F32)
                _edge_phase(nc, tc, c2, cfg, 2, nodefeat2, (ell0, ell1),
                            att2_t, bias2_t, own_base_t, None, x3_t, sel_t,
                            pool_ps, identity_t)

                # ---------------- fc + log_softmax
                sb = c2.enter_context(tc.tile_pool(name="fcsb", bufs=1))
                ps2 = c2.enter_context(tc.tile_pool(name="fcps", bufs=1,
                                                    space="PSUM"))
                wfc_t = sb.tile([HC, 2], F16)
                nc.sync.dma_start(out=wfc_t[:], in_=wfcT[:])
                bfc_t = sb.tile([GPC, 2], F32)
                nc.sync.dma_start(out=bfc_t[:], in_=bfc_rep[:])
                pooledT_t = sb.tile([HC, GPC], F16)
                nc.scalar.copy(out=pooledT_t[:], in_=pool_ps[:])
                lg_ps = ps2.tile([GPC, 2], F32)
                nc.tensor.matmul(out=lg_ps[:], lhsT=pooledT_t[:],
                                 rhs=wfc_t[:], start=True, stop=True)
                lg_t = sb.tile([GPC, 2], F32)
                nc.vector.tensor_add(out=lg_t[:], in0=lg_ps[:], in1=bfc_t[:])
                mx_t = sb.tile([GPC, 1], F32)
                nc.vector.tensor_reduce(out=mx_t[:], in_=lg_t[:], axis=AX.X,
                                        op=ALU.max)
                sh_t = sb.tile([GPC, 2], F32)
                nc.vector.tensor_scalar(
                    out=sh_t[:], in0=lg_t[:], scalar1=mx_t[:, 0:1],
                    scalar2=None, op0=ALU.subtract,
                )
                exl_t = sb.tile([GPC, 2], F32)
                nc.scalar.activation(exl_t[:], sh_t[:], ACTF.Exp)
                se_t = sb.tile([GPC, 1], F32)
                nc.vector.tensor_reduce(out=se_t[:], in_=exl_t[:], axis=AX.X,
                                        op=ALU.add)
                ln_t = sb.tile([GPC, 1], F32)
                nc.scalar.activation(ln_t[:], se_t[:], ACTF.Ln)
                out_t = sb.tile([GPC, 2], F32)
                nc.vector.tensor_scalar(
                    out=out_t[:], in0=sh_t[:], scalar1=ln_t[:, 0:1],
                    scalar2=None, op0=ALU.subtract,
                )
                nc.sync.dma_start(out=logits_out[:], in_=out_t[:])

    nc.compile()
    return nc


# =================================================================== driver
_CACHE = {}


def kernel(**inputs) -> np.ndarray:
    cfg, in_maps = _prep(inputs)
    key = _cfg_key(cfg)
    if key not in _CACHE:
        _CACHE[key] = _build(cfg)
    nc = _CACHE[key]
    res = bass_utils.run_bass_kernel_spmd(nc, in_maps, core_ids=list(range(NCORES)))
    out = np.concatenate([res.results[c]["logits"] for c in range(NCORES)], axis=0)
    return out.astype(np.float32)


# revision 22
# speedup vs baseline: 1.2865x; 1.0001x over previous
"""GATv2 graph net (IMDB) Trainium2 kernel — 8-core SPMD, fp16 edge phase.

Architecture (v2):
- dst-partition edges across 8 cores on graph-aligned node ranges; per-core
  degree-sorted padded ELL (rows = destination nodes, slots on the free dim).
- Node features (xl|xr) are stored fp16 in a replicated DRAM table with 256B
  rows addressed by a transposed tiling r = (g%128)*401 + g//128.  Per-slot
  source xl (64B) comes in via SWDGE dma_gather.  int16 gather indices only
  span 32768 rows, so two overlapping windows are used: [0,32768) and
  [18560,51328).  Sources whose row falls in the overlap are assigned per
  destination row to whichever window balances the two slot counts, which
  keeps per-tile ELL padding low (~1.3x vs 1.7x for a blind split).
- Edge compute runs on group-uniform slot counts: G consecutive tiles share
  one K per window so the whole group is processed by ~20 large fp16 vector
  instructions (4D access patterns, 2x DVE mode) instead of ~26 per tile.
  exp() stays f32 (fp16 would overflow); alpha is normalized before the
  weighted message sum so everything downstream of exp is fp16 again.
- ELU's -1 is folded into the next layer's bias (b2' = b2 - W2@1,
  bfc' = bfc - Wfc@1), so the edge phase emits relu(o)+exp(min(o,0)).
- Mean-pool is a matmul: pooled^T = sum_t x3_tile^T @ Sel_tile where Sel has
  1/graph_size at (node row, local graph) — no gather, no transposes, and the
  fc layer consumes pooled^T directly.
- Between convs the per-core x2^T is AllGathered in fp16.
"""

import sys

sys.path.insert(0, "/opt/trn_rl_repo")

import numpy as np

import concourse.bass as bass
import concourse.bacc as bacc
import concourse.tile as tile
from concourse import mybir
from concourse.bass import exact_div
from concourse.masks import make_identity
from concourse import bass_utils

# ---------------------------------------------------------------- constants
N = 50000
E = 1_600_000
H = 2
C = 16
HC = H * C            # 32
G = 512
NCORES = 8
GPC = G // NCORES     # graphs per core = 64
NPC = 6400            # padded nodes per core (50 tiles of 128)
NT = NPC // 128       # node tiles per core = 50
NG = NCORES * NPC     # padded global node count = 51200
TT = NG // 128        # 400 tiles in gid space
TC1 = TT + 1          # row-columns per partition (400 tiles + 1 spare)
ROWS = 128 * TC1      # node table rows = 51328
W1LO = ROWS - 32768   # window1 base row = 18560
SENT = 60 * TC1 + 400 # sentinel row (spare column, inside the overlap) = 24460+400
BIG = 49152.0         # fp16-safe saturation value for sentinel xl
BUD = 64              # group budget: T*(K0+K1) <= BUD
TMAX = 12             # max tiles per group
F32 = mybir.dt.float32
F16 = mybir.dt.float16
I32 = mybir.dt.int32
I16 = mybir.dt.int16
AX = mybir.AxisListType
ALU = mybir.AluOpType
ACTF = mybir.ActivationFunctionType


def _r_of_gid(g):
    """node-table row for gid: transposed tiling so per-partition rows of one
    core are consecutive (xr indirect read) and node-phase writes batch."""
    return (g % 128) * TC1 + g // 128


def _wrap16(flat):
    """dma_gather index layout: flat i -> [16*g + i%16, i//16], replicated
    across the 8 Q7-core partition groups."""
    flat = np.asarray(flat, np.int16)
    n = len(flat)
    assert n % 16 == 0
    arr = np.empty((128, n // 16), np.int16)
    blk = flat.reshape(n // 16, 16).T
    for g in range(8):
        arr[g * 16 : (g + 1) * 16, :] = blk
    return arr


# ================================================================ host prep
def _prep(inputs):
    ei = np.asarray(inputs["edge_index"])
    src = ei[0].astype(np.int64)
    dst = ei[1].astype(np.int64)
    batch = np.asarray(inputs["batch"]).astype(np.int64)
    rand_feat = np.asarray(inputs["rand_feat"], dtype=np.float32).reshape(-1)

    deg = (np.bincount(src, minlength=N) + np.bincount(dst, minlength=N)).astype(
        np.float32
    )
    ddeg = (np.bincount(dst, minlength=N) + 1).astype(np.int64)  # + self loop

    # graph-aligned core boundaries
    bounds = np.searchsorted(batch, np.arange(0, G + 1, GPC))
    assert bounds[0] == 0 and bounds[-1] == N
    ncs = np.diff(bounds)
    assert ncs.max() <= NPC, ncs.max()

    # per-core degree-sorted node order; gid = core*NPC + rank
    gid = np.empty(N, np.int64)
    for c in range(NCORES):
        lo, hi = bounds[c], bounds[c + 1]
        order = np.argsort(ddeg[lo:hi], kind="stable") + lo
        gid[order] = c * NPC + np.arange(hi - lo)

    # edges (+self loops)
    src_sl = np.concatenate([src, np.arange(N, dtype=np.int64)])
    dst_sl = np.concatenate([dst, np.arange(N, dtype=np.int64)])
    EP = len(src_sl)
    sr = _r_of_gid(gid[src_sl])          # source node-table row
    dgid = gid[dst_sl]

    # ---- window assignment with per-dst balancing over the overlap band
    forced1 = sr >= 32768
    forced0 = sr < W1LO
    flexm = (~forced0) & (~forced1)
    f0 = np.bincount(dgid[forced0], minlength=NG)
    f1 = np.bincount(dgid[forced1], minlength=NG)
    fx = np.bincount(dgid[flexm], minlength=NG)
    degg = f0 + f1 + fx
    quota0 = np.clip((f1 + fx - f0 + 1) // 2, 0, fx)  # flex slots -> window0
    fill0 = f0 + quota0
    fill1 = degg - fill0

    # order edges by dst gid; rank flex edges within each dst
    eorder = np.argsort(dgid, kind="stable")
    dg_s = dgid[eorder]
    sr_s = sr[eorder]
    flex_s = flexm[eorder]
    half_s = forced1[eorder].astype(np.int64)
    fidx = np.nonzero(flex_s)[0]
    dgf = dg_s[fidx]                      # sorted (dg_s sorted)
    frank = np.arange(len(fidx)) - np.searchsorted(dgf, dgf, side="left")
    half_s[fidx] = (frank >= quota0[dgf]).astype(np.int64)

    # slot index within (dst, half); slots ordered by source row so the
    # gather's descriptors walk ascending HBM addresses (row-buffer locality)
    key2 = dg_s * 2 + half_s
    k2o = np.lexsort((sr_s, key2))
    ks_start = np.searchsorted(key2[k2o], np.arange(NG * 2))
    slot = np.empty(EP, np.int64)
    slot[k2o] = np.arange(EP) - ks_start[key2[k2o]]

    # per-tile K per window (max over cores and rows)
    F0 = fill0.reshape(NCORES, NT, 128)
    F1 = fill1.reshape(NCORES, NT, 128)
    K0t = F0.max(axis=(0, 2)).astype(np.int64)
    K1t = F1.max(axis=(0, 2)).astype(np.int64)

    # greedy grouping: T consecutive tiles share (K0, K1); T*(K0+K1) <= BUD
    groups = []
    t = 0
    while t < NT:
        T = 1
        k0 = max(1, int(K0t[t])); k1 = max(1, int(K1t[t]))
        while t + T < NT and T < TMAX:
            nk0 = max(k0, int(K0t[t + T])); nk1 = max(k1, int(K1t[t + T]))
            if (T + 1) * (nk0 + nk1) > BUD:
                break
            T += 1; k0, k1 = nk0, nk1
        groups.append((t, T, k0, k1))
        t += T

    colbase = np.zeros((2, NT), np.int64)   # slot-column start per (half, tile)
    goff = [0, 0]
    groups2 = []
    for (t0, T, k0, k1) in groups:
        assert k0 >= 1 and k1 >= 1
        groups2.append((t0, T, k0, k1, goff[0], goff[1]))
        for tt in range(T):
            colbase[0, t0 + tt] = goff[0] + tt * k0
            colbase[1, t0 + tt] = goff[1] + tt * k1
        goff[0] += T * k0
        goff[1] += T * k1
    groups = groups2
    ncols = [int(goff[0]), int(goff[1])]
    n_slots = (ncols[0] + ncols[1]) * 128

    # ---- ELL index arrays (wrapped int16), pads -> sentinel
    core_of = dg_s // NPC
    j = dg_s % NPC
    tt_ = j // 128
    p_ = j % 128
    ell = []
    for h in range(2):
        base = W1LO if h == 1 else 0
        sent_rel = SENT - base
        flat = np.full((NCORES, ncols[h] * 128), sent_rel, np.int64)
        selm = np.nonzero(half_s == h)[0]
        ghk = [(g[2] if h == 0 else g[3]) for g in groups]
        # group K per tile for assert
        pos = (colbase[h][tt_[selm]] + slot[selm]) * 128 + p_[selm]
        flat[core_of[selm], pos] = sr_s[selm] - base
        assert flat.max() <= 32767 and flat.min() >= 0
        ell.append(flat)

    # ---- pooling selection matrix (1/graph_size at (row, local graph))
    gsz = np.bincount(batch, minlength=G).astype(np.float64)
    sel = np.zeros((NCORES, 128, NT * GPC), np.float16)
    # node n sits at gid[n] = c*NPC + j ; local graph = batch[n] - c*GPC
    nc_ = gid // NPC
    nj = gid % NPC
    npp = nj % 128
    ntt = nj // 128
    lg = batch - nc_ * GPC
    sel[nc_, npp, ntt * GPC + lg] = (1.0 / np.maximum(gsz[batch], 1.0)).astype(
        np.float16
    )

    # ---- x1 features in gid-column order [2, NG]
    x1feat = np.zeros((2, NG), np.float32)
    x1feat[0, gid] = deg
    x1feat[1, gid] = rand_feat

    # ---- own-row base for the per-conv xr indirect read
    own_base = np.empty((NCORES, 128, 1), np.int32)
    for c in range(NCORES):
        own_base[c, :, 0] = np.arange(128) * TC1 + c * NT

    # ---- packed weights
    def f32(x):
        return np.asarray(x, dtype=np.float32)

    W1l, W1r = f32(inputs["W1l"]), f32(inputs["W1r"])
    b1 = np.concatenate([f32(inputs["b1l"]), f32(inputs["b1r"])])
    W2l, W2r = f32(inputs["W2l"]), f32(inputs["W2r"])
    # fold ELU's -1 (x2_stored = x2_true + 1) into conv2 bias
    b2 = np.concatenate(
        [f32(inputs["b2l"]) - W2l.sum(axis=1), f32(inputs["b2r"]) - W2r.sum(axis=1)]
    )
    att1 = f32(inputs["att1"]).reshape(HC)
    att2 = f32(inputs["att2"]).reshape(HC)
    bias1 = f32(inputs["bias1"])
    bias2 = f32(inputs["bias2"])
    Wfc, bfc = f32(inputs["Wfc"]), f32(inputs["bfc"])
    bfc = bfc - Wfc.sum(axis=1)  # fold x3_stored = x3_true + 1

    wpack1 = np.concatenate([W1l.T, W1r.T], axis=1)
    wpack1[0, :] += b1
    wpack2 = np.concatenate([W2l.T, W2r.T], axis=1)
    wpack2 = np.concatenate([wpack2, b2[None, :]], axis=0).astype(np.float16)

    shared = dict(
        x1feat=x1feat,
        wpack1=wpack1,
        wpack2=wpack2,
        att1_rep=np.tile(att1[None, :], (128, 1)).astype(np.float16),
        att2_rep=np.tile(att2[None, :], (128, 1)).astype(np.float16),
        bias1_rep=np.tile(bias1[None, :], (128, 1)).astype(np.float32),
        bias2_rep=np.tile(bias2[None, :], (128, 1)).astype(np.float32),
        padrow1=(-BIG * np.sign(att1)[None, :]).astype(np.float16),
        padrow2=(-BIG * np.sign(att2)[None, :]).astype(np.float16),
        wfcT=Wfc.T.astype(np.float16).copy(),
        bfc_rep=np.tile(bfc[None, :], (GPC, 1)).astype(np.float32),
    )
    in_maps = []
    for c in range(NCORES):
        m = dict(shared)
        m["ell0"] = _wrap16(ell[0][c])
        m["ell1"] = _wrap16(ell[1][c])
        m["sel"] = np.ascontiguousarray(sel[c])
        m["own_base"] = np.ascontiguousarray(own_base[c])
        in_maps.append(m)

    cfg = dict(groups=tuple(groups), ncols=(ncols[0], ncols[1]), n_slots=n_slots)
    return cfg, in_maps


def _cfg_key(cfg):
    return cfg["groups"]


# ============================================================= device build
def dma_gather_raw(gp, out_ap, in_ap, idxs_ap, num_idxs, elem_size, elem_step,
                   queue_num=0):
    """dma_gather without the elem_size%256 assert (64B elems work on HW)."""
    stride_bytes = elem_step * mybir.dt.size(in_ap.dtype)
    stride_bytes_256 = exact_div(stride_bytes, 256)
    _in_ap = gp.lower_ap_dma(in_ap, for_custom_bir_dma=True)
    _idxs_ap = gp.lower_ap(idxs_ap)
    _out_ap = gp.lower_ap(out_ap)
    return gp.add_instruction(
        mybir.InstDMAGatherAnt(
            name=gp.bass.get_next_instruction_name(),
            ins=[*_in_ap, _idxs_ap, gp.lower_val_access(gp.to_reg(num_idxs))],
            outs=[_out_ap],
            transpose=False,
            num_idxs=num_idxs,
            elem_size=elem_size,
            stride_bytes_256=stride_bytes_256,
            gen_mode=0,
            single_packet=False,
            queue_num=queue_num,
        )
    )


def _node_phase_conv1(nc, tc, ctx, x1feat, wp1_t, nodefeat, padrow1):
    A = 16
    with tc.tile_pool(name="n1sb", bufs=3) as sb, tc.tile_pool(
        name="n1ps", bufs=3, space="PSUM"
    ) as ps:
        for g0 in range(0, TT, A):
            x1t = sb.tile([3, A * 128], F32, tag="x1t")
            nc.gpsimd.memset(x1t[0:1, :], 1.0)
            nc.sync.dma_start(
                out=x1t[1:3, :], in_=x1feat[:, g0 * 128 : (g0 + A) * 128]
            )
            stage = sb.tile([128, A * 64], F16, tag="n1stage")
            for a4 in range(0, A, 4):
                pt = ps.tile([128, 4 * 64], F32, tag="n1psum")
                for a in range(4):
                    nc.tensor.matmul(
                        out=pt[:, a * 64 : (a + 1) * 64],
                        lhsT=x1t[:, (a4 + a) * 128 : (a4 + a + 1) * 128],
                        rhs=wp1_t[:],
                        start=True,
                        stop=True,
                    )
                nc.scalar.copy(out=stage[:, a4 * 64 : (a4 + 4) * 64], in_=pt[:])
            dst = bass.AP(
                nodefeat[:].tensor,
                g0 * 128,
                [[TC1 * 128, 128], [128, A], [1, 64]],
            )
            nc.sync.dma_start(out=dst, in_=stage[:])
        pr = sb.tile([1, HC], F16, tag="n1pad")
        nc.sync.dma_start(out=pr[:], in_=padrow1[:])
        nc.sync.dma_start(out=nodefeat[SENT : SENT + 1, 0:HC], in_=pr[:])


def _node_phase_conv2(nc, tc, ctx, agout, wp2_t, nodefeat, padrow2):
    A = 10
    with tc.tile_pool(name="n2sb", bufs=3) as sb, tc.tile_pool(
        name="n2ps", bufs=3, space="PSUM"
    ) as ps:
        for c8 in range(NCORES):
            for t0 in range(0, NT, A):
                lh = sb.tile([33, A * 128], F16, tag="n2lhs")
                src_ap = bass.AP(
                    agout[:].tensor,
                    c8 * 33 * NPC + t0 * 128,
                    [[NPC, 33], [1, A * 128]],
                )
                nc.sync.dma_start(out=lh[:], in_=src_ap)
                stage = sb.tile([128, A * 64], F16, tag="n2stage")
                for a5 in range(0, A, 5):
                    pt = ps.tile([128, 5 * 64], F32, tag="n2psum")
                    for a in range(5):
                        nc.tensor.matmul(
                            out=pt[:, a * 64 : (a + 1) * 64],
                            lhsT=lh[:, (a5 + a) * 128 : (a5 + a + 1) * 128],
                            rhs=wp2_t[:],
                            start=True,
                            stop=True,
                        )
                    nc.scalar.copy(out=stage[:, a5 * 64 : (a5 + 5) * 64],
                                   in_=pt[:])
                dst = bass.AP(
                    nodefeat[:].tensor,
                    (c8 * NT + t0) * 128,
                    [[TC1 * 128, 128], [128, A], [1, 64]],
                )
                nc.sync.dma_start(out=dst, in_=stage[:])
        pr = sb.tile([1, HC], F16, tag="n2pad")
        nc.sync.dma_start(out=pr[:], in_=padrow2[:])
        nc.sync.dma_start(out=nodefeat[SENT : SENT + 1, 0:HC], in_=pr[:])


def _edge_phase(nc, tc, ctx, cfg, conv, nodefeat, ell_d, att_t, bias_t, own_base_t,
                x2t_stage, x3_t, sel_t, pool_ps, identity_t):
    """One GATv2 conv aggregation over group-uniform ELL tiles.

    conv=1: writes transposed x2 into x2t_stage [33, NPC] (fp16).
    conv=2: writes x3 tiles into x3_t [128, NT*HC] (fp16) and accumulates the
            pooling matmul into pool_ps [HC, GPC].
    """
    groups = cfg["groups"]
    MAXH = max(max(T * k0, T * k1) for (_, T, k0, k1, _, _) in groups)
    MAXT = max(T for (_, T, k0, k1, _, _) in groups)
    sbg = ctx.enter_context(tc.tile_pool(name=f"e{conv}sbg", bufs=6))
    sbi = ctx.enter_context(tc.tile_pool(name=f"e{conv}sbi", bufs=6))
    sbz = ctx.enter_context(tc.tile_pool(name=f"e{conv}sbz", bufs=2))
    sbm = ctx.enter_context(tc.tile_pool(name=f"e{conv}sbm", bufs=3))
    sbs = ctx.enter_context(tc.tile_pool(name=f"e{conv}sbs", bufs=2))
    ps = ctx.enter_context(tc.tile_pool(name=f"e{conv}ps", bufs=2, space="PSUM"))

    # all own-node rows (xl|xr fp16, full 256B rows) for this core
    xrp = ctx.enter_context(tc.tile_pool(name=f"e{conv}xr", bufs=1))
    xrall = xrp.tile([128, NT * 128], F16, name=f"xrall{conv}")
    nc.gpsimd.indirect_dma_start(
        out=xrall[:],
        out_offset=None,
        in_=nodefeat[:],
        in_offset=bass.IndirectOffsetOnAxis(ap=own_base_t[:], axis=0),
        element_offset=0,
    )

    def ap4(t, off, dims):
        return bass.AP(t[:].tensor, t[:].offset + off, [t[:].ap[0]] + dims)

    pdim = xrall[:].ap[0]
    NGRP = len(groups)
    for gi in range(NGRP - 1, -1, -1):          # big groups first
        t0, T, gk0, gk1, off0, off1 = groups[gi]
        gk = (gk0, gk1)
        goff = (off0, off1)
        gbuf = [None, None]
        for h in range(2):
            nidx = 128 * T * gk[h]
            i0 = goff[h] * 128
            idx_t = sbi.tile([128, nidx // 16], I16, tag=f"idx{h}",
                             padded_shape=[128, MAXH * 8])
            nc.sync.dma_start(
                out=idx_t[:], in_=ell_d[h][:, i0 // 16 : (i0 + nidx) // 16]
            )
            gbuf[h] = sbg.tile(
                [128, T * gk[h] * HC], F16, tag=f"g{h}", name=f"gbuf{h}",
                padded_shape=[128, MAXH * HC],
            )
            base = W1LO if h == 1 else 0
            dma_gather_raw(
                nc.gpsimd,
                gbuf[h][:].rearrange("p (n e) -> p n e", e=HC),
                nodefeat[base : base + 32768, 0:HC],
                idx_t[:],
                nidx,
                HC,
                128,
                queue_num=0,
            )

        # per-half chain: z -> lrelu -> *att -> score -> exp -> msg (frees gbuf)
        ph = [None, None]
        den_h = [None, None]
        nh = [None, None]
        for h in range(2):
            TK = T * gk[h]
            z_t = sbz.tile([128, TK * HC], F16, tag="z",
                           padded_shape=[128, MAXH * HC])
            nc.vector.tensor_tensor(
                out=z_t[:],
                in0=gbuf[h][:],
                in1=bass.AP(xrall[:].tensor, xrall[:].offset + t0 * 128 + HC,
                            [pdim, [128, T], [0, gk[h]], [1, HC]]),
                op=ALU.add,
            )
            e_t = sbz.tile([128, TK * HC], F16, tag="e",
                           padded_shape=[128, MAXH * HC])
            nc.vector.scalar_tensor_tensor(
                out=e_t[:], in0=z_t[:], scalar=0.2, in1=z_t[:],
                op0=ALU.mult, op1=ALU.max,
            )
            ea_t = sbz.tile([128, TK * HC], F16, tag="ea",
                            padded_shape=[128, MAXH * HC])
            nc.vector.tensor_tensor(
                out=ea_t[:],
                in0=e_t[:],
                in1=bass.AP(att_t[:].tensor, att_t[:].offset,
                            [att_t[:].ap[0], [0, TK], [1, HC]]),
                op=ALU.mult,
            )
            s_t = sbs.tile([128, TK * H], F32, tag="s",
                           padded_shape=[128, MAXH * H])
            nc.vector.tensor_reduce(
                out=s_t[:],
                in_=ap4(ea_t, 0, [[HC, TK], [C, H], [1, C]]),
                axis=AX.X,
                op=ALU.add,
            )
            sc_t = sbs.tile([128, TK * H], F32, tag="sc",
                            padded_shape=[128, MAXH * H])
            nc.vector.tensor_scalar(
                out=sc_t[:], in0=s_t[:], scalar1=-80.0, scalar2=None,
                op0=ALU.max,
            )
            p_t = sbs.tile([128, TK * H], F32, tag="p", name=f"p{h}",
                           padded_shape=[128, MAXH * H])
            nc.scalar.activation(p_t[:], sc_t[:], ACTF.Exp)
            ph[h] = p_t
            den_h[h] = sbs.tile([128, T * H], F32, tag="den", name=f"den{h}",
                                padded_shape=[128, MAXT * H])
            nc.vector.tensor_reduce(
                out=den_h[h][:],
                in_=ap4(p_t, 0, [[gk[h] * H, T], [1, H], [H, gk[h]]]),
                axis=AX.X,
                op=ALU.add,
            )
        den_t = sbs.tile([128, T * H], F32, tag="dens",
                         padded_shape=[128, MAXT * H])
        nc.vector.tensor_add(out=den_t[:], in0=den_h[0][:], in1=den_h[1][:])
        rden_t = sbs.tile([128, T * H], F32, tag="rden",
                          padded_shape=[128, MAXT * H])
        nc.vector.reciprocal(out=rden_t[:], in_=den_t[:])
        # alpha = p*rden (fp16, per half) ; msg = xl*alpha ; num_h = sum_k msg
        for h in range(2):
            TK = T * gk[h]
            al_t = sbs.tile([128, TK * H], F16, tag="al", name=f"al{h}",
                            padded_shape=[128, MAXH * H])
            nc.vector.tensor_tensor(
                out=al_t[:],
                in0=ph[h][:],
                in1=ap4(rden_t, 0, [[H, T], [0, gk[h]], [1, H]]),
                op=ALU.mult,
            )
            msg = sbm.tile([128, TK * HC], F16, tag="m",
                           padded_shape=[128, MAXH * HC])
            nc.vector.tensor_tensor(
                out=msg[:],
                in0=gbuf[h][:],
                in1=ap4(al_t, 0, [[H, TK], [1, H], [0, C]]),
                op=ALU.mult,
            )
            nh[h] = sbs.tile([128, T * HC], F32, tag=f"n{h}", name=f"nh{h}",
                             padded_shape=[128, MAXT * HC])
            nc.vector.tensor_reduce(
                out=nh[h][:],
                in_=ap4(msg, 0, [[gk[h] * HC, T], [1, HC], [HC, gk[h]]]),
                axis=AX.X,
                op=ALU.add,
            )
        num_t = sbs.tile([128, T * HC], F32, tag="num",
                         padded_shape=[128, MAXT * HC])
        nc.vector.tensor_add(out=num_t[:], in0=nh[0][:], in1=nh[1][:])
        # o = num + bias ; x2 = relu(o) + exp(clamp(min(o,0),-80)) (+1 folded)
        o_t = sbs.tile([128, T * HC], F32, tag="o",
                       padded_shape=[128, MAXT * HC])
        nc.vector.tensor_tensor(
            out=o_t[:],
            in0=num_t[:],
            in1=bass.AP(bias_t[:].tensor, bias_t[:].offset,
                        [bias_t[:].ap[0], [0, T], [1, HC]]),
            op=ALU.add,
        )
        mn_t = sbs.tile([128, T * HC], F32, tag="mn",
                        padded_shape=[128, MAXT * HC])
        nc.vector.tensor_scalar(
            out=mn_t[:], in0=o_t[:], scalar1=0.0, scalar2=-80.0,
            op0=ALU.min, op1=ALU.max,
        )
        ex_t = sbs.tile([128, T * HC], F32, tag="ex",
                        padded_shape=[128, MAXT * HC])
        nc.scalar.activation(ex_t[:], mn_t[:], ACTF.Exp)
        if conv == 1:
            x2g = sbs.tile([128, T * HC], F16, tag="x2g",
                           padded_shape=[128, MAXT * HC])
            nc.vector.scalar_tensor_tensor(
                out=x2g[:], in0=o_t[:], scalar=0.0, in1=ex_t[:],
                op0=ALU.max, op1=ALU.add,
            )
            # transpose into x2t_stage [33, NPC] via PE, 4 tiles per transpose
            for ch0 in range(0, T, 4):
                cw = min(4, T - ch0) * HC
                tp = ps.tile([128, 128], F16, tag="tp")
                nc.tensor.transpose(
                    out=tp[0:cw, :],
                    in_=x2g[:, ch0 * HC : ch0 * HC + cw],
                    identity=identity_t[:],
                )
                for k in range((cw) // HC):
                    tt = t0 + ch0 + k
                    nc.scalar.copy(
                        out=x2t_stage[0:HC, tt * 128 : (tt + 1) * 128],
                        in_=tp[k * HC : (k + 1) * HC, :],
                    )
        else:
            nc.vector.scalar_tensor_tensor(
                out=x3_t[:, t0 * HC : (t0 + T) * HC], in0=o_t[:], scalar=0.0,
                in1=ex_t[:], op0=ALU.max, op1=ALU.add,
            )
            first_exec = groups[-1][0]
            last_exec = groups[0][0] + groups[0][1] - 1
            for k in range(T):
                tt = t0 + k
                nc.tensor.matmul(
                    out=pool_ps[:],
                    lhsT=x3_t[:, tt * HC : (tt + 1) * HC],
                    rhs=sel_t[:, tt * GPC : (tt + 1) * GPC],
                    start=(tt == first_exec),
                    stop=(tt == last_exec),
                )


def _build(cfg):
    nc = bacc.Bacc("TRN2", target_bir_lowering=False, debug=False,
                   num_devices=NCORES)
    ncol0, ncol1 = cfg["ncols"]

    x1feat = nc.dram_tensor("x1feat", [2, NG], F32, kind="ExternalInput").ap()
    ell0 = nc.dram_tensor("ell0", [128, ncol0 * 8], I16, kind="ExternalInput").ap()
    ell1 = nc.dram_tensor("ell1", [128, ncol1 * 8], I16, kind="ExternalInput").ap()
    sel = nc.dram_tensor("sel", [128, NT * GPC], F16, kind="ExternalInput").ap()
    own_base = nc.dram_tensor("own_base", [128, 1], I32, kind="ExternalInput").ap()
    wpack1 = nc.dram_tensor("wpack1", [3, 64], F32, kind="ExternalInput").ap()
    wpack2 = nc.dram_tensor("wpack2", [33, 64], F16, kind="ExternalInput").ap()
    att1_rep = nc.dram_tensor("att1_rep", [128, HC], F16, kind="ExternalInput").ap()
    att2_rep = nc.dram_tensor("att2_rep", [128, HC], F16, kind="ExternalInput").ap()
    bias1_rep = nc.dram_tensor("bias1_rep", [128, HC], F32, kind="ExternalInput").ap()
    bias2_rep = nc.dram_tensor("bias2_rep", [128, HC], F32, kind="ExternalInput").ap()
    padrow1 = nc.dram_tensor("padrow1", [1, HC], F16, kind="ExternalInput").ap()
    padrow2 = nc.dram_tensor("padrow2", [1, HC], F16, kind="ExternalInput").ap()
    wfcT = nc.dram_tensor("wfcT", [HC, 2], F16, kind="ExternalInput").ap()
    bfc_rep = nc.dram_tensor("bfc_rep", [GPC, 2], F32, kind="ExternalInput").ap()
    logits_out = nc.dram_tensor("logits", [GPC, 2], F32, kind="ExternalOutput").ap()

    with tile.TileContext(nc) as tc:
        from contextlib import ExitStack

        with ExitStack() as top:
            dram = top.enter_context(tc.tile_pool(name="dram", bufs=1, space="DRAM"))
            nodefeat1 = dram.tile([ROWS, 128], F16)
            nodefeat2 = dram.tile([ROWS, 128], F16)
            agin = dram.tile([33, NPC], F16)
            agout = dram.tile([NCORES * 33, NPC], F16)

            consts = top.enter_context(tc.tile_pool(name="consts", bufs=1))
            wp1_t = consts.tile([3, 64], F32)
            nc.sync.dma_start(out=wp1_t[:], in_=wpack1[:])
            wp2_t = consts.tile([33, 64], F16)
            nc.sync.dma_start(out=wp2_t[:], in_=wpack2[:])
            att1_t = consts.tile([128, HC], F16)
            nc.sync.dma_start(out=att1_t[:], in_=att1_rep[:])
            att2_t = consts.tile([128, HC], F16)
            nc.sync.dma_start(out=att2_t[:], in_=att2_rep[:])
            bias1_t = consts.tile([128, HC], F32)
            nc.sync.dma_start(out=bias1_t[:], in_=bias1_rep[:])
            bias2_t = consts.tile([128, HC], F32)
            nc.sync.dma_start(out=bias2_t[:], in_=bias2_rep[:])
            idf32 = consts.tile([128, 128], F32)
            make_identity(nc, idf32[:])
            identity_t = consts.tile([128, 128], F16)
            nc.vector.tensor_copy(out=identity_t[:], in_=idf32[:])
            own_base_t = consts.tile([128, 1], I32)
            nc.sync.dma_start(out=own_base_t[:], in_=own_base[:])
            sel_t = consts.tile([128, NT * GPC], F16)
            nc.sync.dma_start(out=sel_t[:], in_=sel[:])

            # ---------------- conv1
            _node_phase_conv1(nc, tc, top, x1feat, wp1_t, nodefeat1, padrow1)

            with ExitStack() as c1:
                stage_pool = c1.enter_context(tc.tile_pool(name="x2tst", bufs=1))
                x2t_stage = stage_pool.tile([33, NPC], F16)
                nc.gpsimd.memset(x2t_stage[32:33, :], 1.0)
                _edge_phase(nc, tc, c1, cfg, 1, nodefeat1, (ell0, ell1),
                            att1_t, bias1_t, own_base_t, x2t_stage, None, None,
                            None, identity_t)
                nc.sync.dma_start(out=agin[:], in_=x2t_stage[:])

            nc.gpsimd.collective_compute(
                "AllGather",
                ALU.bypass,
                replica_groups=[list(range(NCORES))],
                ins=[agin[:].opt()],
                outs=[agout[:].opt()],
            )

            # ---------------- conv2
            _node_phase_conv2(nc, tc, top, agout, wp2_t, nodefeat2, padrow2)

            with ExitStack() as c2:
                x3p = c2.enter_context(tc.tile_pool(name="x3p", bufs=1))
                x3_t = x3p.tile([128, NT * HC], F16)
                pps = c2.enter_context(tc.tile_pool(name="poolps", bufs=1,
                                                    space="PSUM"))
                pool_ps = pps.tile([HC, GPC], F32)
                _edge_phase(nc, tc, c2, cfg, 2, nodefeat2, (ell0, ell1),
                            att2_t, bias2_t, own_base_t, None, x3_t, sel_t,
                            pool_ps, identity_t)

                # ---------------- fc + log_softmax
                sb = c2.enter_context(tc.tile_pool(name="fcsb", bufs=1))
                ps2 = c2.enter_context(tc.tile_pool(name="fcps", bufs=1,
                                                    space="PSUM"))
                wfc_t = sb.tile([HC, 2], F16)
                nc.sync.dma_start(out=wfc_t[:], in_=wfcT[:])
                bfc_t = sb.tile([GPC, 2], F32)
                nc.sync.dma_start(out=bfc_t[:], in_=bfc_rep[:])
                pooledT_t = sb.tile([HC, GPC], F16)
                nc.scalar.copy(out=pooledT_t[:], in_=pool_ps[:])
                lg_ps = ps2.tile([GPC, 2], F32)
                nc.tensor.matmul(out=lg_ps[:], lhsT=pooledT_t[:],
                                 rhs=wfc_t[:], start=True, stop=True)
                lg_t = sb.tile([GPC, 2], F32)
                nc.vector.tensor_add(out=lg_t[:], in0=lg_ps[:], in1=bfc_t[:])
                mx_t = sb.tile([GPC, 1], F32)
                nc.vector.tensor_reduce(out=mx_t[:], in_=lg_t[:], axis=AX.X,
                                        op=ALU.max)
                sh_t = sb.tile([GPC, 2], F32)
                nc.vector.tensor_scalar(
                    out=sh_t[:], in0=lg_t[:], scalar1=mx_t[:, 0:1],
                    scalar2=None, op0=ALU.subtract,
                )
                exl_t = sb.tile([GPC, 2], F32)
                nc.scalar.activation(exl_t[:], sh_t[:], ACTF.Exp)
                se_t = sb.tile([GPC, 1], F32)
                nc.vector.tensor_reduce(out=se_t[:], in_=exl_t[:], axis=AX.X,
                                        op=ALU.add)
                ln_t = sb.tile([GPC, 1], F32)
                nc.scalar.activation(ln_t[:], se_t[:], ACTF.Ln)
                out_t = sb.tile([GPC, 2], F32)
                nc.vector.tensor_scalar(
                    out=out_t[:], in0=sh_t[:], scalar1=ln_t[:, 0:1],
                    scalar2=None, op0=ALU.subtract,
                )
                nc.sync.dma_start(out=logits_out[:], in_=out_t[:])

    nc.compile()
    return nc


# =================================================================== driver
_CACHE = {}


def kernel(**inputs) -> np.ndarray:
    cfg, in_maps = _prep(inputs)
    key = _cfg_key(cfg)
    if key not in _CACHE:
        _CACHE[key] = _build(cfg)
    nc = _CACHE[key]
    res = bass_utils.run_bass_kernel_spmd(nc, in_maps, core_ids=list(range(NCORES)))
    out = np.concatenate([res.results[c]["logits"] for c in range(NCORES)], axis=0)
    return out.astype(np.float32)
